# revision 5
# baseline (speedup 1.0000x reference)
"""Trainium2 Bass kernel for nn_EnhancedCoordAtt — v3.

vs baseline (97us -> ~90us):
  - h-gate x2-replication instead of full ACT expansion: ACT writes
    ah2[p,cb,h,0:2] (256 elems, ~0.4us instead of 27us); mulB's in1 AP
    [(cb h) stride-2][w_hi 0-stride][w_lo unit-2] keeps the innermost
    dim unit-stride/2-elem, which is all the DVE 2x_1p mode needs
    (HW-verified 4426ns for [128,8192], same as the full expansion).
  - col-pooling levels L1-L3 (8-row partial sums) moved to the PE as
    identity-lhsT accumulated matmuls into PSUM (16 mm of f=512 per
    sample, contiguous-64 rhs inner dim, fp32-exact), ACT evacuates and
    casts to fp16; DVE only does the last two levels (-3.3us/sample).
    The row tree stays on DVE: its reduction axis IS the contiguous w,
    so PE rhs slices would have 8-elem strided runs (measured 2x cost).
  - PE col phases hoisted ahead of conv/gate matmuls in program order so
    the PE never waits on a sample's DVE round-trip before starting the
    next col block (this PE drag taxed all concurrent DVE muls ~18%).
  - loads/stores striped over both HWDGE rings (single-ring DMA was
    measured to slow concurrent DVE ops by ~18%).
  - s0 trees chunked (cb0 h-halves first) for earliest start; last
    sample's muls/stores chunked per (cb, h-half) for a short tail.
  - gate prefuse for s0/s1: g = aw*ah2 is combined during the DVE's
    load-bound idle windows (~27-33us), so their gate application in the
    packed phase is a single x*g pass instead of two.
All shapes hardcoded to the nn_EnhancedCoordAtt_78855599555233 spec.
"""

import numpy as np

N, C, H, W = 32, 256, 64, 64
MIP = 8
N_CORES = 8
S = N // N_CORES
HW = H * W
T = H + W
BN_EPS = 1e-5

_CACHE = {}


def _legalize_waits(nc, mybir, max_keep=1):
    f = nc.m.functions[0]
    for blk in f.blocks:
        out, changed = [], False
        for inst in blk.instructions:
            si = inst.sync_info
            t = type(inst).__name__
            if (si is not None and len(si.on_wait) > max_keep
                    and t != "InstEventSemaphore"):
                waits = list(si.on_wait)
                for j, w in enumerate(waits[:-max_keep]):
                    ev = mybir.InstEventSemaphore(
                        name=f"{inst.name}_xw{j}", ins=[], outs=[])
                    ev.engine = inst.engine
                    ev.sync_info = mybir.SyncInfo(on_wait=[w], on_update=[])
                    out.append(ev)
                inst.sync_info = mybir.SyncInfo(
                    on_wait=waits[-max_keep:], on_update=list(si.on_update))
                changed = True
            out.append(inst)
        if changed:
            blk.instructions = out


def _build_program(legalize=True, sim_compat=False):
    import concourse.bass as bass
    import concourse.tile as tile
    import concourse.mybir as mybir
    from contextlib import ExitStack

    f16 = mybir.dt.float16
    f32 = mybir.dt.float32
    i32 = mybir.dt.int32
    nc = bass.Bass()

    xs = nc.declare_dram_parameter("xs", [S, C, H, W], f16, isOutput=False)
    w2 = nc.declare_dram_parameter("w2", [128, 4, 3, MIP], f16, isOutput=False)
    bias2 = nc.declare_dram_parameter("bias2", [MIP, 1], f32, isOutput=False)
    gwv = nc.declare_dram_parameter("gwv", [MIP, 1], f32, isOutput=False)
    gbv = nc.declare_dram_parameter("gbv", [MIP, 1], f32, isOutput=False)
    wh = nc.declare_dram_parameter("wh", [MIP, C], f16, isOutput=False)
    ww = nc.declare_dram_parameter("ww", [MIP, C], f16, isOutput=False)
    bh = nc.declare_dram_parameter("bh", [128, 2], f32, isOutput=False)
    bw = nc.declare_dram_parameter("bw", [128, 2], f32, isOutput=False)
    idn = nc.declare_dram_parameter("idn", [128, 128], f16, isOutput=False)
    out = nc.declare_dram_parameter("out", [S, C, H, W], f16, isOutput=True)

    with tile.TileContext(nc) as tc, ExitStack() as ctx:
        ctx.enter_context(nc.allow_low_precision(reason="2e-2 tolerance, fp16 path"))
        red = nc.vector
        Sig = mybir.ActivationFunctionType.Sigmoid
        Copy = mybir.ActivationFunctionType.Copy

        singles = ctx.enter_context(tc.tile_pool(name="singles", bufs=1))
        xpool = ctx.enter_context(tc.tile_pool(name="xin", bufs=4))
        ypool = ctx.enter_context(tc.tile_pool(name="yall", bufs=4))
        small = ctx.enter_context(tc.tile_pool(name="small", bufs=4))
        tpool = ctx.enter_context(tc.tile_pool(name="tree", bufs=2))
        cpool = ctx.enter_context(tc.tile_pool(name="colp", bufs=4))
        apool = ctx.enter_context(tc.tile_pool(name="attn", bufs=4))
        gpool = ctx.enter_context(tc.tile_pool(name="gfuse", bufs=1))
        pspool = ctx.enter_context(tc.tile_pool(name="ps", bufs=2, space="PSUM"))
        psgate = ctx.enter_context(tc.tile_pool(name="psg", bufs=1, space="PSUM"))
        pscol = ctx.enter_context(tc.tile_pool(name="psc", bufs=4, space="PSUM"))

        # ---- identity matrix for PE copy/accumulate matmuls (host param) ----
        ident = singles.tile([128, 128], f16)
        nc.gpsimd.dma_start(out=ident, in_=idn[:, :])

        # ---- params on gpsimd SWDGE (never queues behind x traffic) ----
        w2sb = singles.tile([128, 4, 3, MIP], f16)
        nc.gpsimd.dma_start(out=w2sb, in_=w2[:, :, :, :])
        bias2sb = singles.tile([MIP, 1], f32)
        nc.gpsimd.dma_start(out=bias2sb, in_=bias2[:, :])
        gwsb = singles.tile([MIP, 1], f32)
        nc.gpsimd.dma_start(out=gwsb, in_=gwv[:, :])
        gbsb = singles.tile([MIP, 1], f32)
        nc.gpsimd.dma_start(out=gbsb, in_=gbv[:, :])
        whsb = singles.tile([MIP, C], f16)
        nc.gpsimd.dma_start(out=whsb, in_=wh[:, :])
        wwsb = singles.tile([MIP, C], f16)
        nc.gpsimd.dma_start(out=wwsb, in_=ww[:, :])
        bhsb = singles.tile([128, 2], f32)
        nc.gpsimd.dma_start(out=bhsb, in_=bh[:, :])
        bwsb = singles.tile([128, 2], f32)
        nc.gpsimd.dma_start(out=bwsb, in_=bw[:, :])

        # ---- x loads, striped across both rings ----
        all_xts = []
        for s in range(S):
            xt = xpool.tile([128, 2 * HW], f16, tag="xt")
            all_xts.append(xt)
            src_full = xs[s].rearrange("(cb c) h w -> c cb (h w)", cb=2)
            if s == 0:
                # per-cb halves on both rings so cb0 completes earliest
                for cb in range(2):
                    for hh in range(2):
                        eng = nc.sync if hh == 0 else nc.scalar
                        sl = slice(cb * HW + hh * (HW // 2),
                                   cb * HW + (hh + 1) * (HW // 2))
                        eng.dma_start(
                            out=xt[:, sl],
                            in_=src_full[:, cb, hh * (HW // 2):(hh + 1) * (HW // 2)])
            else:
                for cb in range(2):
                    eng = nc.sync if cb == 0 else nc.scalar
                    eng.dma_start(
                        out=xt[:, cb * HW:(cb + 1) * HW], in_=src_full[:, cb])

        def row_tree_half(xt, y_all, cb, i):
            """Row-half sums for one (cb, h-half) chunk (earliest start)."""
            xh = xt[:, cb * HW + i * (HW // 2): cb * HW + (i + 1) * (HW // 2)]
            rv = xh.rearrange("p (hj w) -> p hj w", w=W // 2)
            r1 = tpool.tile([128, H, 16], f16, tag="hr1")
            red.tensor_add(out=r1, in0=rv[:, :, 0:16], in1=rv[:, :, 16:32])
            r2 = tpool.tile([128, H, 8], f16, tag="hr2")
            red.tensor_add(out=r2, in0=r1[:, :, 0:8], in1=r1[:, :, 8:16])
            r3 = tpool.tile([128, H, 4], f16, tag="hr3")
            red.tensor_add(out=r3, in0=r2[:, :, 0:4], in1=r2[:, :, 4:8])
            r4 = tpool.tile([128, H, 2], f16, tag="hr4")
            red.tensor_add(out=r4, in0=r3[:, :, 0:2], in1=r3[:, :, 2:4])
            rv4 = r4.rearrange("p (h j) a -> p j h a", j=2)
            red.tensor_add(
                out=y_all.rearrange("p (j c) t -> p c j t", j=2)[
                    :, cb, :, i * (H // 2):(i + 1) * (H // 2)],
                in0=rv4[:, :, :, 0], in1=rv4[:, :, :, 1])

        def row_tree_cb(xt, y_all, cb):
            """Row-half sums for one cb block (s0 path: starts on cb0)."""
            xc = xt[:, cb * HW:(cb + 1) * HW]
            rv = xc.rearrange("p (hj w) -> p hj w", w=W // 2)
            r1 = tpool.tile([128, 2 * H, 16], f16, tag="r1")
            red.tensor_add(out=r1, in0=rv[:, :, 0:16], in1=rv[:, :, 16:32])
            r2 = tpool.tile([128, 2 * H, 8], f16, tag="r2")
            red.tensor_add(out=r2, in0=r1[:, :, 0:8], in1=r1[:, :, 8:16])
            r3 = tpool.tile([128, 2 * H, 4], f16, tag="r3")
            red.tensor_add(out=r3, in0=r2[:, :, 0:4], in1=r2[:, :, 4:8])
            r4 = tpool.tile([128, 2 * H, 2], f16, tag="r4")
            red.tensor_add(out=r4, in0=r3[:, :, 0:2], in1=r3[:, :, 2:4])
            rv4 = r4.rearrange("p (h j) a -> p j h a", j=2)
            red.tensor_add(
                out=y_all.rearrange("p (j c) t -> p c j t", j=2)[:, cb, :, 0:H],
                in0=rv4[:, :, :, 0], in1=rv4[:, :, :, 1])

        def row_tree_joint(xt, y_all):
            rv = xt.rearrange("p (cb hj w) -> p cb hj w", cb=2, w=W // 2)
            r1 = tpool.tile([128, 2, 2 * H, 16], f16, tag="jr1")
            red.tensor_add(out=r1, in0=rv[:, :, :, 0:16], in1=rv[:, :, :, 16:32])
            r2 = tpool.tile([128, 2, 2 * H, 8], f16, tag="jr2")
            red.tensor_add(out=r2, in0=r1[:, :, :, 0:8], in1=r1[:, :, :, 8:16])
            r3 = tpool.tile([128, 2, 2 * H, 4], f16, tag="jr3")
            red.tensor_add(out=r3, in0=r2[:, :, :, 0:4], in1=r2[:, :, :, 4:8])
            r4 = tpool.tile([128, 2, 2 * H, 2], f16, tag="jr4")
            red.tensor_add(out=r4, in0=r3[:, :, :, 0:2], in1=r3[:, :, :, 2:4])
            rv4 = r4.rearrange("p cb (h j) a -> p cb j h a", j=2)
            red.tensor_add(
                out=y_all.rearrange("p (j c) t -> p c j t", j=2)[:, :, :, 0:H],
                in0=rv4[:, :, :, :, 0], in1=rv4[:, :, :, :, 1])

        def col_pe_mm(xt):
            """Col L1-L3 (8-row partials) on PE + ACT evac/cast, per cb."""
            c3es = []
            for cb in range(2):
                psc = pscol.tile([128, 8, W], f32, tag="psc")
                for r in range(8):
                    base = xt[:, cb * HW + r * W:]
                    rhs = bass.AP(tensor=base.tensor, offset=base.offset,
                                  ap=[base.ap[0], [8 * W, 8], [1, W]])
                    nc.tensor.matmul(out=psc, lhsT=ident[:, :], rhs=rhs,
                                     start=(r == 0), stop=(r == 7))
                c3e = cpool.tile([128, 8, W], f16, tag="c3e")
                nc.scalar.activation(out=c3e, in_=psc, func=Copy,
                                     bias=0.0, scale=1.0)
                c3es.append(c3e)
            return c3es

        def col_tail(c3es, y_all):
            """Last two col levels on DVE."""
            for cb in range(2):
                c3e = c3es[cb]
                c4 = cpool.tile([128, 2, 2, W], f16, tag="c4")
                cv = c3e.rearrange("p (i m) w -> p i m w", i=2)
                red.tensor_add(out=c4, in0=cv[:, :, 0:2], in1=cv[:, :, 2:4])
                red.tensor_add(
                    out=y_all.rearrange("p (i c) t -> p i c t", i=2)[:, :, cb, H:T],
                    in0=c4[:, :, 0, :], in1=c4[:, :, 1, :])

        def gates(y_all):
            psy = pspool.tile([MIP, T], f32, tag="psy")
            order = [(0, 1)] + [(g, k) for g in range(4) for k in range(3)
                                if (g, k) != (0, 1)]
            for idx, (g, k) in enumerate(order):
                lhsT = w2sb[:, g, k, :]
                if k == 1:
                    o_sl, i_sl = slice(0, T), slice(0, T)
                elif k == 0:
                    o_sl, i_sl = slice(2, T), slice(0, T - 2)
                else:
                    o_sl, i_sl = slice(0, T - 2), slice(2, T)
                nc.tensor.matmul(
                    out=psy[:, o_sl], lhsT=lhsT, rhs=y_all[:, g, i_sl],
                    start=(idx == 0), stop=(idx == len(order) - 1))

            ya0 = small.tile([MIP, T], f32, tag="ya0")
            nc.vector.tensor_scalar_add(out=ya0, in0=psy, scalar1=bias2sb[:, :])
            ysg = small.tile([MIP, T], f32, tag="ysg")
            nc.scalar.activation(out=ysg, in_=ya0, func=Sig, bias=0.0, scale=1.0)
            ya = small.tile([MIP, T], f32, tag="ya")
            red.tensor_mul(out=ya, in0=ya0, in1=ysg)
            ysum = small.tile([MIP, 1], f32, tag="ysum")
            red.reduce_sum(out=ysum, in_=ya, axis=mybir.AxisListType.X)
            se = small.tile([MIP, 1], f32, tag="se")
            nc.scalar.activation(out=se, in_=ysum, func=Sig,
                                 bias=gbsb[:, :], scale=gwsb[:, :])
            yg = small.tile([MIP, T], f16, tag="yg")
            nc.vector.tensor_scalar_mul(out=yg, in0=ya, scalar1=se[:, :])

            ah2 = apool.tile([128, 2, H, 2], f16, tag="ah2")
            aw2 = apool.tile([128, 2, W], f16, tag="aw2")
            for cb in range(2):
                psa = psgate.tile([128, H], f32, tag="psa")
                nc.tensor.matmul(
                    out=psa, lhsT=whsb[:, cb * 128:(cb + 1) * 128],
                    rhs=yg[:, 0:H], start=True, stop=True)
                pa = psa[:, :]
                pab = bass.AP(tensor=pa.tensor, offset=pa.offset,
                              ap=[pa.ap[0], pa.ap[1], [0, 2]])
                nc.scalar.activation(out=ah2[:, cb], in_=pab, func=Sig,
                                     bias=bhsb[:, cb:cb + 1], scale=1.0)
                psb = psgate.tile([128, W], f32, tag="psb")
                nc.tensor.matmul(
                    out=psb, lhsT=wwsb[:, cb * 128:(cb + 1) * 128],
                    rhs=yg[:, H:T], start=True, stop=True)
                nc.scalar.activation(out=aw2[:, cb], in_=psb, func=Sig,
                                     bias=bwsb[:, cb:cb + 1], scale=1.0)
            return ah2, aw2

        def gate_prefuse(ah2, aw2, s):
            """Combine g = aw * ah2 into a full fp16 tile during DVE idle
            (load-bound) windows; the mul phase then needs ONE pass."""
            g = gpool.tile([128, 2 * HW], f16, tag=f"g{s}")
            aa = ah2[:, :, :, :]
            a = aw2[:, :, :]
            for cb in range(2):
                gout = bass.AP(tensor=g.tensor, offset=g[:, cb * HW:].offset,
                               ap=[g[:, :].ap[0], [W, H], [2, W // 2], [1, 2]])
                gin0 = bass.AP(tensor=a.tensor, offset=aw2[:, cb, :].offset,
                               ap=[a.ap[0], [0, H], [2, W // 2], [1, 2]])
                gin1 = bass.AP(tensor=aa.tensor, offset=ah2[:, cb].offset,
                               ap=[aa.ap[0], [2, H], [0, W // 2], [1, 2]])
                red.tensor_mul(out=gout, in0=gin0, in1=gin1)
            return g

        def muls_fused_store(s, xt, g):
            """out = x * g in one DVE pass, then store."""
            red.tensor_mul(out=xt[:, :], in0=xt[:, :], in1=g[:, :])
            ost = out[s].rearrange("(cb c) h w -> c cb (h w)", cb=2)
            nc.sync.dma_start(out=ost[:, 0], in_=xt[:, 0:HW])
            nc.scalar.dma_start(out=ost[:, 1], in_=xt[:, HW:2 * HW])

        def muls_and_store(s, xt, ah2, aw2):
            xv = xt.rearrange("p (cb h w) -> p cb h w", cb=2, w=W)
            a = aw2[:, :, :]
            awb = bass.AP(tensor=a.tensor, offset=a.offset,
                          ap=[a.ap[0], a.ap[1], [0, H], a.ap[2]])
            aa = ah2[:, :, :, :]
            ahf = bass.AP(tensor=aa.tensor, offset=aa.offset,
                          ap=[aa.ap[0], [2, 128], [0, W // 2], [1, 2]])
            xf = bass.AP(tensor=xt.tensor, offset=xt[:, :].offset,
                         ap=[xt[:, :].ap[0], [W, 2 * H], [2, W // 2], [1, 2]])
            ost = out[s].rearrange("(cb c) h w -> c cb (h w)", cb=2)
            if s < S - 1:
                red.tensor_mul(out=xv, in0=xv, in1=awb)
                red.tensor_mul(out=xf, in0=xf, in1=ahf)
                nc.sync.dma_start(out=ost[:, 0], in_=xt[:, 0:HW])
                nc.scalar.dma_start(out=ost[:, 1], in_=xt[:, HW:2 * HW])
            else:
                # last sample: per-(cb, h-half) muls with eighth-tile stores
                for cb in range(2):
                    awc = bass.AP(tensor=a.tensor, offset=aw2[:, cb, :].offset,
                                  ap=[a.ap[0], [0, H // 2], a.ap[2]])
                    for hh in range(2):
                        off = cb * HW + hh * (HW // 2)
                        xvc = xt[:, off:off + HW // 2].rearrange(
                            "p (h w) -> p h w", w=W)
                        red.tensor_mul(out=xvc, in0=xvc, in1=awc)
                        xfc = bass.AP(tensor=xt.tensor,
                                      offset=xt[:, off:].offset,
                                      ap=[xt[:, :].ap[0], [W, H // 2],
                                          [2, W // 2], [1, 2]])
                        ahc = bass.AP(tensor=aa.tensor,
                                      offset=ah2[:, cb, hh * (H // 2):].offset,
                                      ap=[aa.ap[0], [2, H // 2],
                                          [0, W // 2], [1, 2]])
                        red.tensor_mul(out=xfc, in0=xfc, in1=ahc)
                        nq = 4 if (cb == 1 and hh == 1) else 2
                        for qq in range(nq):
                            eng = nc.sync if qq % 2 == 0 else nc.scalar
                            csz = (HW // 2) // nq
                            osl = slice(hh * (HW // 2) + qq * csz,
                                        hh * (HW // 2) + (qq + 1) * csz)
                            eng.dma_start(
                                out=ost[:, cb, osl],
                                in_=xt[:, off + qq * csz:off + (qq + 1) * csz])

        # ---------- schedule: trees in sample order, then muls ----------
        y_tiles = []
        for s in range(S):
            yt = ypool.tile([128, 4, T], f16, tag=f"y{s}")
            y_tiles.append(yt)

        gates_out = [None] * S
        # PE col phases hoisted ahead of conv/gates so the PE never waits
        # on a sample's DVE round-trip before starting the next col block.
        row_tree_half(all_xts[0], y_tiles[0], 0, 0)
        row_tree_half(all_xts[0], y_tiles[0], 0, 1)
        c3_0 = col_pe_mm(all_xts[0])
        row_tree_cb(all_xts[0], y_tiles[0], 1)
        c3_1 = col_pe_mm(all_xts[1])
        col_tail(c3_0, y_tiles[0])
        gates_out[0] = gates(y_tiles[0])
        c3_2 = col_pe_mm(all_xts[2])
        g0 = gate_prefuse(*gates_out[0], 0)
        row_tree_joint(all_xts[1], y_tiles[1])
        col_tail(c3_1, y_tiles[1])
        gates_out[1] = gates(y_tiles[1])
        c3_3 = col_pe_mm(all_xts[3])
        g1 = gate_prefuse(*gates_out[1], 1)
        row_tree_joint(all_xts[2], y_tiles[2])
        col_tail(c3_2, y_tiles[2])
        gates_out[2] = gates(y_tiles[2])
        muls_fused_store(0, all_xts[0], g0)
        row_tree_joint(all_xts[3], y_tiles[3])
        col_tail(c3_3, y_tiles[3])
        gates_out[3] = gates(y_tiles[3])
        muls_fused_store(1, all_xts[1], g1)
        muls_and_store(2, all_xts[2], *gates_out[2])
        muls_and_store(3, all_xts[3], *gates_out[3])

    if legalize:
        import concourse.mybir as mybir
        _legalize_waits(nc, mybir)
    return nc


def _prep_params(conv1_w, conv1_b, bn_gamma, bn_beta, bn_mean, bn_var,
                 gate_w, gate_b, convh_w, convh_b, convw_w, convw_b):
    f32 = np.float32
    bnscale = (np.asarray(bn_gamma, f32)
               / np.sqrt(np.asarray(bn_var, f32) + BN_EPS)).astype(f32)
    Wc = np.asarray(conv1_w, f32)[:, :, :, 1]
    s_ci = np.where(np.arange(3 * C) < C, 1.0 / W, 2.0 / W).astype(f32)
    W2 = (Wc * s_ci[None, :, None] * bnscale[:, None, None]).astype(f32)
    bias2 = ((np.asarray(conv1_b, f32) - np.asarray(bn_mean, f32)) * bnscale
             + np.asarray(bn_beta, f32)).astype(f32)
    W6 = W2.reshape(MIP, 6, 128, 3)
    W4 = np.stack([W6[:, 2 + gp] + W6[:, gp % 2] for gp in range(4)], axis=1)
    w2 = np.ascontiguousarray(W4.transpose(2, 1, 3, 0)).astype(np.float16)
    gw = np.full((MIP, 1), float(gate_w) / T, f32)
    gb = np.full((MIP, 1), float(gate_b), f32)
    wh = np.ascontiguousarray(np.asarray(convh_w, np.float16).T)
    ww = np.ascontiguousarray(np.asarray(convw_w, np.float16).T)
    bh = np.ascontiguousarray(np.asarray(convh_b, f32).reshape(2, 128).T)
    bw = np.ascontiguousarray(np.asarray(convw_b, f32).reshape(2, 128).T)
    idn = np.ascontiguousarray(np.eye(128, dtype=np.float16))
    return dict(w2=w2, bias2=bias2.reshape(MIP, 1), gwv=gw, gbv=gb,
                wh=wh, ww=ww, bh=bh, bw=bw, idn=idn)


def kernel(**inputs):
    import sys
    if "/opt/trn_rl_repo" not in sys.path:
        sys.path.insert(0, "/opt/trn_rl_repo")
    from concourse.bass_utils import run_bass_kernel_spmd

    x = np.asarray(inputs["x"], np.float32).astype(np.float16)
    params = _prep_params(
        inputs["conv1_w"], inputs["conv1_b"], inputs["bn_gamma"],
        inputs["bn_beta"], inputs["bn_mean"], inputs["bn_var"],
        inputs["gate_w"], inputs["gate_b"], inputs["convh_w"],
        inputs["convh_b"], inputs["convw_w"], inputs["convw_b"])

    if "nc" not in _CACHE:
        _CACHE["nc"] = _build_program()
    nc = _CACHE["nc"]

    in_maps = [
        {"xs": np.ascontiguousarray(x[i * S:(i + 1) * S]), **params}
        for i in range(N_CORES)
    ]
    res = run_bass_kernel_spmd(nc, in_maps, core_ids=list(range(N_CORES)))
    out = np.concatenate([r["out"] for r in res.results], axis=0)
    return out.astype(np.float32)


# revision 6
# speedup vs baseline: 1.0143x; 1.0143x over previous
"""Trainium2 Bass kernel for nn_EnhancedCoordAtt — v3.

vs baseline (97us -> ~90us):
  - h-gate x2-replication instead of full ACT expansion: ACT writes
    ah2[p,cb,h,0:2] (256 elems, ~0.4us instead of 27us); mulB's in1 AP
    [(cb h) stride-2][w_hi 0-stride][w_lo unit-2] keeps the innermost
    dim unit-stride/2-elem, which is all the DVE 2x_1p mode needs
    (HW-verified 4426ns for [128,8192], same as the full expansion).
  - col-pooling levels L1-L3 (8-row partial sums) moved to the PE as
    identity-lhsT accumulated matmuls into PSUM (16 mm of f=512 per
    sample, contiguous-64 rhs inner dim, fp32-exact), ACT evacuates and
    casts to fp16; DVE only does the last two levels (-3.3us/sample).
    The row tree stays on DVE: its reduction axis IS the contiguous w,
    so PE rhs slices would have 8-elem strided runs (measured 2x cost).
  - PE col phases hoisted ahead of conv/gate matmuls in program order so
    the PE never waits on a sample's DVE round-trip before starting the
    next col block (this PE drag taxed all concurrent DVE muls ~18%).
  - loads/stores striped over both HWDGE rings (single-ring DMA was
    measured to slow concurrent DVE ops by ~18%).
  - s0 trees chunked (cb0 h-halves first) for earliest start; last
    sample's muls/stores chunked per (cb, h-half) for a short tail.
  - gate prefuse for s0/s1: g = aw*ah2 is combined during the DVE's
    load-bound idle windows (~27-33us), so their gate application in the
    packed phase is a single x*g pass instead of two.
All shapes hardcoded to the nn_EnhancedCoordAtt_78855599555233 spec.
"""

import numpy as np

N, C, H, W = 32, 256, 64, 64
MIP = 8
N_CORES = 8
S = N // N_CORES
HW = H * W
T = H + W
BN_EPS = 1e-5

_CACHE = {}


def _legalize_waits(nc, mybir, max_keep=1):
    f = nc.m.functions[0]
    for blk in f.blocks:
        out, changed = [], False
        for inst in blk.instructions:
            si = inst.sync_info
            t = type(inst).__name__
            if (si is not None and len(si.on_wait) > max_keep
                    and t != "InstEventSemaphore"):
                waits = list(si.on_wait)
                for j, w in enumerate(waits[:-max_keep]):
                    ev = mybir.InstEventSemaphore(
                        name=f"{inst.name}_xw{j}", ins=[], outs=[])
                    ev.engine = inst.engine
                    ev.sync_info = mybir.SyncInfo(on_wait=[w], on_update=[])
                    out.append(ev)
                inst.sync_info = mybir.SyncInfo(
                    on_wait=waits[-max_keep:], on_update=list(si.on_update))
                changed = True
            out.append(inst)
        if changed:
            blk.instructions = out


def _build_program(legalize=True, sim_compat=False):
    import concourse.bass as bass
    import concourse.tile as tile
    import concourse.mybir as mybir
    from contextlib import ExitStack

    f16 = mybir.dt.float16
    f32 = mybir.dt.float32
    i32 = mybir.dt.int32
    nc = bass.Bass()

    xs = nc.declare_dram_parameter("xs", [S, C, H, W], f16, isOutput=False)
    w2 = nc.declare_dram_parameter("w2", [128, 4, 3, MIP], f16, isOutput=False)
    bias2 = nc.declare_dram_parameter("bias2", [MIP, 1], f32, isOutput=False)
    gwv = nc.declare_dram_parameter("gwv", [MIP, 1], f32, isOutput=False)
    gbv = nc.declare_dram_parameter("gbv", [MIP, 1], f32, isOutput=False)
    wh = nc.declare_dram_parameter("wh", [MIP, C], f16, isOutput=False)
    ww = nc.declare_dram_parameter("ww", [MIP, C], f16, isOutput=False)
    bh = nc.declare_dram_parameter("bh", [128, 2], f32, isOutput=False)
    bw = nc.declare_dram_parameter("bw", [128, 2], f32, isOutput=False)
    idn = nc.declare_dram_parameter("idn", [128, 128], f16, isOutput=False)
    out = nc.declare_dram_parameter("out", [S, C, H, W], f16, isOutput=True)

    with tile.TileContext(nc) as tc, ExitStack() as ctx:
        ctx.enter_context(nc.allow_low_precision(reason="2e-2 tolerance, fp16 path"))
        red = nc.vector
        Sig = mybir.ActivationFunctionType.Sigmoid
        Copy = mybir.ActivationFunctionType.Copy

        singles = ctx.enter_context(tc.tile_pool(name="singles", bufs=1))
        xpool = ctx.enter_context(tc.tile_pool(name="xin", bufs=4))
        ypool = ctx.enter_context(tc.tile_pool(name="yall", bufs=4))
        small = ctx.enter_context(tc.tile_pool(name="small", bufs=4))
        tpool = ctx.enter_context(tc.tile_pool(name="tree", bufs=2))
        cpool = ctx.enter_context(tc.tile_pool(name="colp", bufs=4))
        apool = ctx.enter_context(tc.tile_pool(name="attn", bufs=4))
        gpool = ctx.enter_context(tc.tile_pool(name="gfuse", bufs=1))
        pspool = ctx.enter_context(tc.tile_pool(name="ps", bufs=2, space="PSUM"))
        psgate = ctx.enter_context(tc.tile_pool(name="psg", bufs=1, space="PSUM"))
        pscol = ctx.enter_context(tc.tile_pool(name="psc", bufs=4, space="PSUM"))

        # ---- identity matrix for PE copy/accumulate matmuls (host param) ----
        ident = singles.tile([128, 128], f16)
        nc.gpsimd.dma_start(out=ident, in_=idn[:, :])

        # ---- params on gpsimd SWDGE (never queues behind x traffic) ----
        w2sb = singles.tile([128, 4, 3, MIP], f16)
        nc.gpsimd.dma_start(out=w2sb, in_=w2[:, :, :, :])
        bias2sb = singles.tile([MIP, 1], f32)
        nc.gpsimd.dma_start(out=bias2sb, in_=bias2[:, :])
        gwsb = singles.tile([MIP, 1], f32)
        nc.gpsimd.dma_start(out=gwsb, in_=gwv[:, :])
        gbsb = singles.tile([MIP, 1], f32)
        nc.gpsimd.dma_start(out=gbsb, in_=gbv[:, :])
        whsb = singles.tile([MIP, C], f16)
        nc.gpsimd.dma_start(out=whsb, in_=wh[:, :])
        wwsb = singles.tile([MIP, C], f16)
        nc.gpsimd.dma_start(out=wwsb, in_=ww[:, :])
        bhsb = singles.tile([128, 2], f32)
        nc.gpsimd.dma_start(out=bhsb, in_=bh[:, :])
        bwsb = singles.tile([128, 2], f32)
        nc.gpsimd.dma_start(out=bwsb, in_=bw[:, :])

        # ---- x loads, striped across both rings ----
        all_xts = []
        for s in range(S):
            xt = xpool.tile([128, 2 * HW], f16, tag="xt")
            all_xts.append(xt)
            src_full = xs[s].rearrange("(cb c) h w -> c cb (h w)", cb=2)
            if s == 0:
                # per-cb halves on both rings so cb0 completes earliest
                for cb in range(2):
                    for hh in range(2):
                        eng = nc.sync if hh == 0 else nc.scalar
                        sl = slice(cb * HW + hh * (HW // 2),
                                   cb * HW + (hh + 1) * (HW // 2))
                        eng.dma_start(
                            out=xt[:, sl],
                            in_=src_full[:, cb, hh * (HW // 2):(hh + 1) * (HW // 2)])
            else:
                for cb in range(2):
                    eng = nc.sync if cb == 0 else nc.scalar
                    eng.dma_start(
                        out=xt[:, cb * HW:(cb + 1) * HW], in_=src_full[:, cb])

        def row_tree_half(xt, y_all, cb, i):
            """Row-half sums for one (cb, h-half) chunk (earliest start)."""
            xh = xt[:, cb * HW + i * (HW // 2): cb * HW + (i + 1) * (HW // 2)]
            rv = xh.rearrange("p (hj w) -> p hj w", w=W // 2)
            r1 = tpool.tile([128, H, 16], f16, tag="hr1")
            red.tensor_add(out=r1, in0=rv[:, :, 0:16], in1=rv[:, :, 16:32])
            r2 = tpool.tile([128, H, 8], f16, tag="hr2")
            red.tensor_add(out=r2, in0=r1[:, :, 0:8], in1=r1[:, :, 8:16])
            r3 = tpool.tile([128, H, 4], f16, tag="hr3")
            red.tensor_add(out=r3, in0=r2[:, :, 0:4], in1=r2[:, :, 4:8])
            r4 = tpool.tile([128, H, 2], f16, tag="hr4")
            red.tensor_add(out=r4, in0=r3[:, :, 0:2], in1=r3[:, :, 2:4])
            rv4 = r4.rearrange("p (h j) a -> p j h a", j=2)
            red.tensor_add(
                out=y_all.rearrange("p (j c) t -> p c j t", j=2)[
                    :, cb, :, i * (H // 2):(i + 1) * (H // 2)],
                in0=rv4[:, :, :, 0], in1=rv4[:, :, :, 1])

        def row_tree_cb(xt, y_all, cb):
            """Row-half sums for one cb block (s0 path: starts on cb0)."""
            xc = xt[:, cb * HW:(cb + 1) * HW]
            rv = xc.rearrange("p (hj w) -> p hj w", w=W // 2)
            r1 = tpool.tile([128, 2 * H, 16], f16, tag="r1")
            red.tensor_add(out=r1, in0=rv[:, :, 0:16], in1=rv[:, :, 16:32])
            r2 = tpool.tile([128, 2 * H, 8], f16, tag="r2")
            red.tensor_add(out=r2, in0=r1[:, :, 0:8], in1=r1[:, :, 8:16])
            r3 = tpool.tile([128, 2 * H, 4], f16, tag="r3")
            red.tensor_add(out=r3, in0=r2[:, :, 0:4], in1=r2[:, :, 4:8])
            r4 = tpool.tile([128, 2 * H, 2], f16, tag="r4")
            red.tensor_add(out=r4, in0=r3[:, :, 0:2], in1=r3[:, :, 2:4])
            rv4 = r4.rearrange("p (h j) a -> p j h a", j=2)
            red.tensor_add(
                out=y_all.rearrange("p (j c) t -> p c j t", j=2)[:, cb, :, 0:H],
                in0=rv4[:, :, :, 0], in1=rv4[:, :, :, 1])

        def row_tree_joint(xt, y_all):
            rv = xt.rearrange("p (cb hj w) -> p cb hj w", cb=2, w=W // 2)
            r1 = tpool.tile([128, 2, 2 * H, 16], f16, tag="jr1")
            red.tensor_add(out=r1, in0=rv[:, :, :, 0:16], in1=rv[:, :, :, 16:32])
            r2 = tpool.tile([128, 2, 2 * H, 8], f16, tag="jr2")
            red.tensor_add(out=r2, in0=r1[:, :, :, 0:8], in1=r1[:, :, :, 8:16])
            r3 = tpool.tile([128, 2, 2 * H, 4], f16, tag="jr3")
            red.tensor_add(out=r3, in0=r2[:, :, :, 0:4], in1=r2[:, :, :, 4:8])
            r4 = tpool.tile([128, 2, 2 * H, 2], f16, tag="jr4")
            red.tensor_add(out=r4, in0=r3[:, :, :, 0:2], in1=r3[:, :, :, 2:4])
            rv4 = r4.rearrange("p cb (h j) a -> p cb j h a", j=2)
            red.tensor_add(
                out=y_all.rearrange("p (j c) t -> p c j t", j=2)[:, :, :, 0:H],
                in0=rv4[:, :, :, :, 0], in1=rv4[:, :, :, :, 1])

        def col_pe_mm(xt):
            """Col L1-L3 (8-row partials) on PE + ACT evac/cast, per cb."""
            c3es = []
            for cb in range(2):
                psc = pscol.tile([128, 8, W], f32, tag="psc")
                for r in range(8):
                    base = xt[:, cb * HW + r * W:]
                    rhs = bass.AP(tensor=base.tensor, offset=base.offset,
                                  ap=[base.ap[0], [8 * W, 8], [1, W]])
                    nc.tensor.matmul(out=psc, lhsT=ident[:, :], rhs=rhs,
                                     start=(r == 0), stop=(r == 7))
                c3e = cpool.tile([128, 8, W], f16, tag="c3e")
                nc.scalar.activation(out=c3e, in_=psc, func=Copy,
                                     bias=0.0, scale=1.0)
                c3es.append(c3e)
            return c3es

        def col_tail(c3es, y_all):
            """Last two col levels on DVE."""
            for cb in range(2):
                c3e = c3es[cb]
                c4 = cpool.tile([128, 2, 2, W], f16, tag="c4")
                cv = c3e.rearrange("p (i m) w -> p i m w", i=2)
                red.tensor_add(out=c4, in0=cv[:, :, 0:2], in1=cv[:, :, 2:4])
                red.tensor_add(
                    out=y_all.rearrange("p (i c) t -> p i c t", i=2)[:, :, cb, H:T],
                    in0=c4[:, :, 0, :], in1=c4[:, :, 1, :])

        def gates(y_all):
            psy = pspool.tile([MIP, T], f32, tag="psy")
            order = [(0, 1)] + [(g, k) for g in range(4) for k in range(3)
                                if (g, k) != (0, 1)]
            for idx, (g, k) in enumerate(order):
                lhsT = w2sb[:, g, k, :]
                if k == 1:
                    o_sl, i_sl = slice(0, T), slice(0, T)
                elif k == 0:
                    o_sl, i_sl = slice(2, T), slice(0, T - 2)
                else:
                    o_sl, i_sl = slice(0, T - 2), slice(2, T)
                nc.tensor.matmul(
                    out=psy[:, o_sl], lhsT=lhsT, rhs=y_all[:, g, i_sl],
                    start=(idx == 0), stop=(idx == len(order) - 1))

            ya0 = small.tile([MIP, T], f32, tag="ya0")
            nc.vector.tensor_scalar_add(out=ya0, in0=psy, scalar1=bias2sb[:, :])
            ysg = small.tile([MIP, T], f32, tag="ysg")
            nc.scalar.activation(out=ysg, in_=ya0, func=Sig, bias=0.0, scale=1.0)
            ya = small.tile([MIP, T], f32, tag="ya")
            red.tensor_mul(out=ya, in0=ya0, in1=ysg)
            ysum = small.tile([MIP, 1], f32, tag="ysum")
            red.reduce_sum(out=ysum, in_=ya, axis=mybir.AxisListType.X)
            se = small.tile([MIP, 1], f32, tag="se")
            nc.scalar.activation(out=se, in_=ysum, func=Sig,
                                 bias=gbsb[:, :], scale=gwsb[:, :])
            yg = small.tile([MIP, T], f16, tag="yg")
            nc.vector.tensor_scalar_mul(out=yg, in0=ya, scalar1=se[:, :])

            ah2 = apool.tile([128, 2, H, 2], f16, tag="ah2")
            aw2 = apool.tile([128, 2, W], f16, tag="aw2")
            for cb in range(2):
                psa = psgate.tile([128, H], f32, tag="psa")
                nc.tensor.matmul(
                    out=psa, lhsT=whsb[:, cb * 128:(cb + 1) * 128],
                    rhs=yg[:, 0:H], start=True, stop=True)
                pa = psa[:, :]
                pab = bass.AP(tensor=pa.tensor, offset=pa.offset,
                              ap=[pa.ap[0], pa.ap[1], [0, 2]])
                nc.scalar.activation(out=ah2[:, cb], in_=pab, func=Sig,
                                     bias=bhsb[:, cb:cb + 1], scale=1.0)
                psb = psgate.tile([128, W], f32, tag="psb")
                nc.tensor.matmul(
                    out=psb, lhsT=wwsb[:, cb * 128:(cb + 1) * 128],
                    rhs=yg[:, H:T], start=True, stop=True)
                nc.scalar.activation(out=aw2[:, cb], in_=psb, func=Sig,
                                     bias=bwsb[:, cb:cb + 1], scale=1.0)
            return ah2, aw2

        def gate_prefuse(ah2, aw2, s):
            """Combine g = aw * ah2 into a full fp16 tile during DVE idle
            (load-bound) windows; the mul phase then needs ONE pass."""
            g = gpool.tile([128, 2 * HW], f16, tag=f"g{s}")
            aa = ah2[:, :, :, :]
            a = aw2[:, :, :]
            for cb in range(2):
                gout = bass.AP(tensor=g.tensor, offset=g[:, cb * HW:].offset,
                               ap=[g[:, :].ap[0], [W, H], [2, W // 2], [1, 2]])
                gin0 = bass.AP(tensor=a.tensor, offset=aw2[:, cb, :].offset,
                               ap=[a.ap[0], [0, H], [2, W // 2], [1, 2]])
                gin1 = bass.AP(tensor=aa.tensor, offset=ah2[:, cb].offset,
                               ap=[aa.ap[0], [2, H], [0, W // 2], [1, 2]])
                red.tensor_mul(out=gout, in0=gin0, in1=gin1)
            return g

        def muls_fused_store(s, xt, g):
            """out = x * g in one DVE pass, then store."""
            red.tensor_mul(out=xt[:, :], in0=xt[:, :], in1=g[:, :])
            ost = out[s].rearrange("(cb c) h w -> c cb (h w)", cb=2)
            nc.sync.dma_start(out=ost[:, 0], in_=xt[:, 0:HW])
            nc.scalar.dma_start(out=ost[:, 1], in_=xt[:, HW:2 * HW])

        def muls_fused_chunk_store(s, xt, g):
            """Last sample: one fused x*g mul per (cb, h-half) chunk, with
            eighth-tile stores so the tail stays short."""
            ost = out[s].rearrange("(cb c) h w -> c cb (h w)", cb=2)
            for cb in range(2):
                for hh in range(2):
                    off = cb * HW + hh * (HW // 2)
                    xc = xt[:, off:off + HW // 2]
                    red.tensor_mul(out=xc, in0=xc, in1=g[:, off:off + HW // 2])
                    nq = 4 if (cb == 1 and hh == 1) else 2
                    for qq in range(nq):
                        eng = nc.sync if qq % 2 == 0 else nc.scalar
                        csz = (HW // 2) // nq
                        osl = slice(hh * (HW // 2) + qq * csz,
                                    hh * (HW // 2) + (qq + 1) * csz)
                        eng.dma_start(
                            out=ost[:, cb, osl],
                            in_=xt[:, off + qq * csz:off + (qq + 1) * csz])

        def muls_and_store(s, xt, ah2, aw2):
            xv = xt.rearrange("p (cb h w) -> p cb h w", cb=2, w=W)
            a = aw2[:, :, :]
            awb = bass.AP(tensor=a.tensor, offset=a.offset,
                          ap=[a.ap[0], a.ap[1], [0, H], a.ap[2]])
            aa = ah2[:, :, :, :]
            ahf = bass.AP(tensor=aa.tensor, offset=aa.offset,
                          ap=[aa.ap[0], [2, 128], [0, W // 2], [1, 2]])
            xf = bass.AP(tensor=xt.tensor, offset=xt[:, :].offset,
                         ap=[xt[:, :].ap[0], [W, 2 * H], [2, W // 2], [1, 2]])
            ost = out[s].rearrange("(cb c) h w -> c cb (h w)", cb=2)
            if s < S - 1:
                red.tensor_mul(out=xv, in0=xv, in1=awb)
                red.tensor_mul(out=xf, in0=xf, in1=ahf)
                nc.sync.dma_start(out=ost[:, 0], in_=xt[:, 0:HW])
                nc.scalar.dma_start(out=ost[:, 1], in_=xt[:, HW:2 * HW])
            else:
                # last sample: per-(cb, h-half) muls with eighth-tile stores
                for cb in range(2):
                    awc = bass.AP(tensor=a.tensor, offset=aw2[:, cb, :].offset,
                                  ap=[a.ap[0], [0, H // 2], a.ap[2]])
                    for hh in range(2):
                        off = cb * HW + hh * (HW // 2)
                        xvc = xt[:, off:off + HW // 2].rearrange(
                            "p (h w) -> p h w", w=W)
                        red.tensor_mul(out=xvc, in0=xvc, in1=awc)
                        xfc = bass.AP(tensor=xt.tensor,
                                      offset=xt[:, off:].offset,
                                      ap=[xt[:, :].ap[0], [W, H // 2],
                                          [2, W // 2], [1, 2]])
                        ahc = bass.AP(tensor=aa.tensor,
                                      offset=ah2[:, cb, hh * (H // 2):].offset,
                                      ap=[aa.ap[0], [2, H // 2],
                                          [0, W // 2], [1, 2]])
                        red.tensor_mul(out=xfc, in0=xfc, in1=ahc)
                        nq = 4 if (cb == 1 and hh == 1) else 2
                        for qq in range(nq):
                            eng = nc.sync if qq % 2 == 0 else nc.scalar
                            csz = (HW // 2) // nq
                            osl = slice(hh * (HW // 2) + qq * csz,
                                        hh * (HW // 2) + (qq + 1) * csz)
                            eng.dma_start(
                                out=ost[:, cb, osl],
                                in_=xt[:, off + qq * csz:off + (qq + 1) * csz])

        # ---------- schedule: trees in sample order, then muls ----------
        y_tiles = []
        for s in range(S):
            yt = ypool.tile([128, 4, T], f16, tag=f"y{s}")
            y_tiles.append(yt)

        gates_out = [None] * S
        # PE col phases hoisted ahead of conv/gates so the PE never waits
        # on a sample's DVE round-trip before starting the next col block.
        row_tree_half(all_xts[0], y_tiles[0], 0, 0)
        row_tree_half(all_xts[0], y_tiles[0], 0, 1)
        c3_0 = col_pe_mm(all_xts[0])
        row_tree_cb(all_xts[0], y_tiles[0], 1)
        c3_1 = col_pe_mm(all_xts[1])
        col_tail(c3_0, y_tiles[0])
        gates_out[0] = gates(y_tiles[0])
        c3_2 = col_pe_mm(all_xts[2])
        g0 = gate_prefuse(*gates_out[0], 0)
        row_tree_joint(all_xts[1], y_tiles[1])
        col_tail(c3_1, y_tiles[1])
        gates_out[1] = gates(y_tiles[1])
        c3_3 = col_pe_mm(all_xts[3])
        g1 = gate_prefuse(*gates_out[1], 1)
        row_tree_joint(all_xts[2], y_tiles[2])
        col_tail(c3_2, y_tiles[2])
        gates_out[2] = gates(y_tiles[2])
        muls_fused_store(0, all_xts[0], g0)
        row_tree_joint(all_xts[3], y_tiles[3])
        col_tail(c3_3, y_tiles[3])
        gates_out[3] = gates(y_tiles[3])
        muls_fused_store(1, all_xts[1], g1)
        g3 = gate_prefuse(*gates_out[3], 3)
        muls_and_store(2, all_xts[2], *gates_out[2])
        muls_fused_chunk_store(3, all_xts[3], g3)

    if legalize:
        import concourse.mybir as mybir
        _legalize_waits(nc, mybir)
    return nc


def _prep_params(conv1_w, conv1_b, bn_gamma, bn_beta, bn_mean, bn_var,
                 gate_w, gate_b, convh_w, convh_b, convw_w, convw_b):
    f32 = np.float32
    bnscale = (np.asarray(bn_gamma, f32)
               / np.sqrt(np.asarray(bn_var, f32) + BN_EPS)).astype(f32)
    Wc = np.asarray(conv1_w, f32)[:, :, :, 1]
    s_ci = np.where(np.arange(3 * C) < C, 1.0 / W, 2.0 / W).astype(f32)
    W2 = (Wc * s_ci[None, :, None] * bnscale[:, None, None]).astype(f32)
    bias2 = ((np.asarray(conv1_b, f32) - np.asarray(bn_mean, f32)) * bnscale
             + np.asarray(bn_beta, f32)).astype(f32)
    W6 = W2.reshape(MIP, 6, 128, 3)
    W4 = np.stack([W6[:, 2 + gp] + W6[:, gp % 2] for gp in range(4)], axis=1)
    w2 = np.ascontiguousarray(W4.transpose(2, 1, 3, 0)).astype(np.float16)
    gw = np.full((MIP, 1), float(gate_w) / T, f32)
    gb = np.full((MIP, 1), float(gate_b), f32)
    wh = np.ascontiguousarray(np.asarray(convh_w, np.float16).T)
    ww = np.ascontiguousarray(np.asarray(convw_w, np.float16).T)
    bh = np.ascontiguousarray(np.asarray(convh_b, f32).reshape(2, 128).T)
    bw = np.ascontiguousarray(np.asarray(convw_b, f32).reshape(2, 128).T)
    idn = np.ascontiguousarray(np.eye(128, dtype=np.float16))
    return dict(w2=w2, bias2=bias2.reshape(MIP, 1), gwv=gw, gbv=gb,
                wh=wh, ww=ww, bh=bh, bw=bw, idn=idn)


def kernel(**inputs):
    import sys
    if "/opt/trn_rl_repo" not in sys.path:
        sys.path.insert(0, "/opt/trn_rl_repo")
    from concourse.bass_utils import run_bass_kernel_spmd

    x = np.asarray(inputs["x"], np.float32).astype(np.float16)
    params = _prep_params(
        inputs["conv1_w"], inputs["conv1_b"], inputs["bn_gamma"],
        inputs["bn_beta"], inputs["bn_mean"], inputs["bn_var"],
        inputs["gate_w"], inputs["gate_b"], inputs["convh_w"],
        inputs["convh_b"], inputs["convw_w"], inputs["convw_b"])

    if "nc" not in _CACHE:
        _CACHE["nc"] = _build_program()
    nc = _CACHE["nc"]

    in_maps = [
        {"xs": np.ascontiguousarray(x[i * S:(i + 1) * S]), **params}
        for i in range(N_CORES)
    ]
    res = run_bass_kernel_spmd(nc, in_maps, core_ids=list(range(N_CORES)))
    out = np.concatenate([r["out"] for r in res.results], axis=0)
    return out.astype(np.float32)


# revision 7
# speedup vs baseline: 1.0191x; 1.0047x over previous
"""Trainium2 Bass kernel for nn_EnhancedCoordAtt — v3.

vs baseline (97us -> ~90us):
  - h-gate x2-replication instead of full ACT expansion: ACT writes
    ah2[p,cb,h,0:2] (256 elems, ~0.4us instead of 27us); mulB's in1 AP
    [(cb h) stride-2][w_hi 0-stride][w_lo unit-2] keeps the innermost
    dim unit-stride/2-elem, which is all the DVE 2x_1p mode needs
    (HW-verified 4426ns for [128,8192], same as the full expansion).
  - col-pooling levels L1-L3 (8-row partial sums) moved to the PE as
    identity-lhsT accumulated matmuls into PSUM (16 mm of f=512 per
    sample, contiguous-64 rhs inner dim, fp32-exact), ACT evacuates and
    casts to fp16; DVE only does the last two levels (-3.3us/sample).
    The row tree stays on DVE: its reduction axis IS the contiguous w,
    so PE rhs slices would have 8-elem strided runs (measured 2x cost).
  - PE col phases hoisted ahead of conv/gate matmuls in program order so
    the PE never waits on a sample's DVE round-trip before starting the
    next col block (this PE drag taxed all concurrent DVE muls ~18%).
  - loads/stores striped over both HWDGE rings (single-ring DMA was
    measured to slow concurrent DVE ops by ~18%).
  - s0 trees chunked (cb0 h-halves first) for earliest start; last
    sample's muls/stores chunked per (cb, h-half) for a short tail.
All shapes hardcoded to the nn_EnhancedCoordAtt_78855599555233 spec.
"""

import numpy as np

N, C, H, W = 32, 256, 64, 64
MIP = 8
N_CORES = 8
S = N // N_CORES
HW = H * W
T = H + W
BN_EPS = 1e-5

_CACHE = {}


def _legalize_waits(nc, mybir, max_keep=1):
    f = nc.m.functions[0]
    for blk in f.blocks:
        out, changed = [], False
        for inst in blk.instructions:
            si = inst.sync_info
            t = type(inst).__name__
            if (si is not None and len(si.on_wait) > max_keep
                    and t != "InstEventSemaphore"):
                waits = list(si.on_wait)
                for j, w in enumerate(waits[:-max_keep]):
                    ev = mybir.InstEventSemaphore(
                        name=f"{inst.name}_xw{j}", ins=[], outs=[])
                    ev.engine = inst.engine
                    ev.sync_info = mybir.SyncInfo(on_wait=[w], on_update=[])
                    out.append(ev)
                inst.sync_info = mybir.SyncInfo(
                    on_wait=waits[-max_keep:], on_update=list(si.on_update))
                changed = True
            out.append(inst)
        if changed:
            blk.instructions = out


def _build_program(legalize=True, sim_compat=False):
    import concourse.bass as bass
    import concourse.tile as tile
    import concourse.mybir as mybir
    from contextlib import ExitStack

    f16 = mybir.dt.float16
    f32 = mybir.dt.float32
    i32 = mybir.dt.int32
    nc = bass.Bass()

    xs = nc.declare_dram_parameter("xs", [S, C, H, W], f16, isOutput=False)
    w2 = nc.declare_dram_parameter("w2", [128, 4, 3, MIP], f16, isOutput=False)
    bias2 = nc.declare_dram_parameter("bias2", [MIP, 1], f32, isOutput=False)
    gwv = nc.declare_dram_parameter("gwv", [MIP, 1], f32, isOutput=False)
    gbv = nc.declare_dram_parameter("gbv", [MIP, 1], f32, isOutput=False)
    wh = nc.declare_dram_parameter("wh", [MIP, C], f16, isOutput=False)
    ww = nc.declare_dram_parameter("ww", [MIP, C], f16, isOutput=False)
    bh = nc.declare_dram_parameter("bh", [128, 2], f32, isOutput=False)
    bw = nc.declare_dram_parameter("bw", [128, 2], f32, isOutput=False)
    idn = nc.declare_dram_parameter("idn", [128, 128], f16, isOutput=False)
    out = nc.declare_dram_parameter("out", [S, C, H, W], f16, isOutput=True)

    with tile.TileContext(nc) as tc, ExitStack() as ctx:
        ctx.enter_context(nc.allow_low_precision(reason="2e-2 tolerance, fp16 path"))
        red = nc.vector
        Sig = mybir.ActivationFunctionType.Sigmoid
        Copy = mybir.ActivationFunctionType.Copy

        singles = ctx.enter_context(tc.tile_pool(name="singles", bufs=1))
        xpool = ctx.enter_context(tc.tile_pool(name="xin", bufs=4))
        ypool = ctx.enter_context(tc.tile_pool(name="yall", bufs=4))
        small = ctx.enter_context(tc.tile_pool(name="small", bufs=4))
        tpool = ctx.enter_context(tc.tile_pool(name="tree", bufs=2))
        cpool = ctx.enter_context(tc.tile_pool(name="colp", bufs=4))
        apool = ctx.enter_context(tc.tile_pool(name="attn", bufs=4))
        gpool = ctx.enter_context(tc.tile_pool(name="gfuse", bufs=1))
        pspool = ctx.enter_context(tc.tile_pool(name="ps", bufs=2, space="PSUM"))
        psgate = ctx.enter_context(tc.tile_pool(name="psg", bufs=1, space="PSUM"))
        pscol = ctx.enter_context(tc.tile_pool(name="psc", bufs=4, space="PSUM"))

        # ---- identity matrix for PE copy/accumulate matmuls (host param) ----
        ident = singles.tile([128, 128], f16)
        nc.gpsimd.dma_start(out=ident, in_=idn[:, :])

        # ---- params on gpsimd SWDGE (never queues behind x traffic) ----
        w2sb = singles.tile([128, 4, 3, MIP], f16)
        nc.gpsimd.dma_start(out=w2sb, in_=w2[:, :, :, :])
        bias2sb = singles.tile([MIP, 1], f32)
        nc.gpsimd.dma_start(out=bias2sb, in_=bias2[:, :])
        gwsb = singles.tile([MIP, 1], f32)
        nc.gpsimd.dma_start(out=gwsb, in_=gwv[:, :])
        gbsb = singles.tile([MIP, 1], f32)
        nc.gpsimd.dma_start(out=gbsb, in_=gbv[:, :])
        whsb = singles.tile([MIP, C], f16)
        nc.gpsimd.dma_start(out=whsb, in_=wh[:, :])
        wwsb = singles.tile([MIP, C], f16)
        nc.gpsimd.dma_start(out=wwsb, in_=ww[:, :])
        bhsb = singles.tile([128, 2], f32)
        nc.gpsimd.dma_start(out=bhsb, in_=bh[:, :])
        bwsb = singles.tile([128, 2], f32)
        nc.gpsimd.dma_start(out=bwsb, in_=bw[:, :])

        # ---- x loads, striped across both rings ----
        all_xts = []
        for s in range(S):
            xt = xpool.tile([128, 2 * HW], f16, tag="xt")
            all_xts.append(xt)
            src_full = xs[s].rearrange("(cb c) h w -> c cb (h w)", cb=2)
            if s == 0:
                # per-cb halves on both rings so cb0 completes earliest
                for cb in range(2):
                    for hh in range(2):
                        eng = nc.sync if hh == 0 else nc.scalar
                        sl = slice(cb * HW + hh * (HW // 2),
                                   cb * HW + (hh + 1) * (HW // 2))
                        eng.dma_start(
                            out=xt[:, sl],
                            in_=src_full[:, cb, hh * (HW // 2):(hh + 1) * (HW // 2)])
            else:
                for cb in range(2):
                    eng = nc.sync if cb == 0 else nc.scalar
                    eng.dma_start(
                        out=xt[:, cb * HW:(cb + 1) * HW], in_=src_full[:, cb])

        def row_tree_half(xt, y_all, cb, i):
            """Row-half sums for one (cb, h-half) chunk (earliest start)."""
            xh = xt[:, cb * HW + i * (HW // 2): cb * HW + (i + 1) * (HW // 2)]
            rv = xh.rearrange("p (hj w) -> p hj w", w=W // 2)
            r1 = tpool.tile([128, H, 16], f16, tag="hr1")
            red.tensor_add(out=r1, in0=rv[:, :, 0:16], in1=rv[:, :, 16:32])
            r2 = tpool.tile([128, H, 8], f16, tag="hr2")
            red.tensor_add(out=r2, in0=r1[:, :, 0:8], in1=r1[:, :, 8:16])
            r3 = tpool.tile([128, H, 4], f16, tag="hr3")
            red.tensor_add(out=r3, in0=r2[:, :, 0:4], in1=r2[:, :, 4:8])
            r4 = tpool.tile([128, H, 2], f16, tag="hr4")
            red.tensor_add(out=r4, in0=r3[:, :, 0:2], in1=r3[:, :, 2:4])
            rv4 = r4.rearrange("p (h j) a -> p j h a", j=2)
            red.tensor_add(
                out=y_all.rearrange("p (j c) t -> p c j t", j=2)[
                    :, cb, :, i * (H // 2):(i + 1) * (H // 2)],
                in0=rv4[:, :, :, 0], in1=rv4[:, :, :, 1])

        def row_tree_cb(xt, y_all, cb):
            """Row-half sums for one cb block (s0 path: starts on cb0)."""
            xc = xt[:, cb * HW:(cb + 1) * HW]
            rv = xc.rearrange("p (hj w) -> p hj w", w=W // 2)
            r1 = tpool.tile([128, 2 * H, 16], f16, tag="r1")
            red.tensor_add(out=r1, in0=rv[:, :, 0:16], in1=rv[:, :, 16:32])
            r2 = tpool.tile([128, 2 * H, 8], f16, tag="r2")
            red.tensor_add(out=r2, in0=r1[:, :, 0:8], in1=r1[:, :, 8:16])
            r3 = tpool.tile([128, 2 * H, 4], f16, tag="r3")
            red.tensor_add(out=r3, in0=r2[:, :, 0:4], in1=r2[:, :, 4:8])
            r4 = tpool.tile([128, 2 * H, 2], f16, tag="r4")
            red.tensor_add(out=r4, in0=r3[:, :, 0:2], in1=r3[:, :, 2:4])
            rv4 = r4.rearrange("p (h j) a -> p j h a", j=2)
            red.tensor_add(
                out=y_all.rearrange("p (j c) t -> p c j t", j=2)[:, cb, :, 0:H],
                in0=rv4[:, :, :, 0], in1=rv4[:, :, :, 1])

        def row_tree_joint(xt, y_all):
            rv = xt.rearrange("p (cb hj w) -> p cb hj w", cb=2, w=W // 2)
            r1 = tpool.tile([128, 2, 2 * H, 16], f16, tag="jr1")
            red.tensor_add(out=r1, in0=rv[:, :, :, 0:16], in1=rv[:, :, :, 16:32])
            r2 = tpool.tile([128, 2, 2 * H, 8], f16, tag="jr2")
            red.tensor_add(out=r2, in0=r1[:, :, :, 0:8], in1=r1[:, :, :, 8:16])
            r3 = tpool.tile([128, 2, 2 * H, 4], f16, tag="jr3")
            red.tensor_add(out=r3, in0=r2[:, :, :, 0:4], in1=r2[:, :, :, 4:8])
            r4 = tpool.tile([128, 2, 2 * H, 2], f16, tag="jr4")
            red.tensor_add(out=r4, in0=r3[:, :, :, 0:2], in1=r3[:, :, :, 2:4])
            rv4 = r4.rearrange("p cb (h j) a -> p cb j h a", j=2)
            red.tensor_add(
                out=y_all.rearrange("p (j c) t -> p c j t", j=2)[:, :, :, 0:H],
                in0=rv4[:, :, :, :, 0], in1=rv4[:, :, :, :, 1])

        def col_pe_mm(xt):
            """Col L1-L3 (8-row partials) on PE + ACT evac/cast, per cb."""
            c3es = []
            for cb in range(2):
                psc = pscol.tile([128, 8, W], f32, tag="psc")
                for r in range(8):
                    base = xt[:, cb * HW + r * W:]
                    rhs = bass.AP(tensor=base.tensor, offset=base.offset,
                                  ap=[base.ap[0], [8 * W, 8], [1, W]])
                    nc.tensor.matmul(out=psc, lhsT=ident[:, :], rhs=rhs,
                                     start=(r == 0), stop=(r == 7))
                c3e = cpool.tile([128, 8, W], f16, tag="c3e")
                nc.scalar.activation(out=c3e, in_=psc, func=Copy,
                                     bias=0.0, scale=1.0)
                c3es.append(c3e)
            return c3es

        def col_tail(c3es, y_all):
            """Last two col levels on DVE."""
            for cb in range(2):
                c3e = c3es[cb]
                c4 = cpool.tile([128, 2, 2, W], f16, tag="c4")
                cv = c3e.rearrange("p (i m) w -> p i m w", i=2)
                red.tensor_add(out=c4, in0=cv[:, :, 0:2], in1=cv[:, :, 2:4])
                red.tensor_add(
                    out=y_all.rearrange("p (i c) t -> p i c t", i=2)[:, :, cb, H:T],
                    in0=c4[:, :, 0, :], in1=c4[:, :, 1, :])

        def gates(y_all):
            psy = pspool.tile([MIP, T], f32, tag="psy")
            order = [(0, 1)] + [(g, k) for g in range(4) for k in range(3)
                                if (g, k) != (0, 1)]
            for idx, (g, k) in enumerate(order):
                lhsT = w2sb[:, g, k, :]
                if k == 1:
                    o_sl, i_sl = slice(0, T), slice(0, T)
                elif k == 0:
                    o_sl, i_sl = slice(2, T), slice(0, T - 2)
                else:
                    o_sl, i_sl = slice(0, T - 2), slice(2, T)
                nc.tensor.matmul(
                    out=psy[:, o_sl], lhsT=lhsT, rhs=y_all[:, g, i_sl],
                    start=(idx == 0), stop=(idx == len(order) - 1))

            ya0 = small.tile([MIP, T], f32, tag="ya0")
            nc.vector.tensor_scalar_add(out=ya0, in0=psy, scalar1=bias2sb[:, :])
            ysg = small.tile([MIP, T], f32, tag="ysg")
            nc.scalar.activation(out=ysg, in_=ya0, func=Sig, bias=0.0, scale=1.0)
            ya = small.tile([MIP, T], f32, tag="ya")
            red.tensor_mul(out=ya, in0=ya0, in1=ysg)
            ysum = small.tile([MIP, 1], f32, tag="ysum")
            red.reduce_sum(out=ysum, in_=ya, axis=mybir.AxisListType.X)
            se = small.tile([MIP, 1], f32, tag="se")
            nc.scalar.activation(out=se, in_=ysum, func=Sig,
                                 bias=gbsb[:, :], scale=gwsb[:, :])
            yg = small.tile([MIP, T], f16, tag="yg")
            nc.vector.tensor_scalar_mul(out=yg, in0=ya, scalar1=se[:, :])

            ah2 = apool.tile([128, 2, H, 2], f16, tag="ah2")
            aw2 = apool.tile([128, 2, W], f16, tag="aw2")
            for cb in range(2):
                psa = psgate.tile([128, H], f32, tag="psa")
                nc.tensor.matmul(
                    out=psa, lhsT=whsb[:, cb * 128:(cb + 1) * 128],
                    rhs=yg[:, 0:H], start=True, stop=True)
                pa = psa[:, :]
                pab = bass.AP(tensor=pa.tensor, offset=pa.offset,
                              ap=[pa.ap[0], pa.ap[1], [0, 2]])
                nc.scalar.activation(out=ah2[:, cb], in_=pab, func=Sig,
                                     bias=bhsb[:, cb:cb + 1], scale=1.0)
                psb = psgate.tile([128, W], f32, tag="psb")
                nc.tensor.matmul(
                    out=psb, lhsT=wwsb[:, cb * 128:(cb + 1) * 128],
                    rhs=yg[:, H:T], start=True, stop=True)
                nc.scalar.activation(out=aw2[:, cb], in_=psb, func=Sig,
                                     bias=bwsb[:, cb:cb + 1], scale=1.0)
            return ah2, aw2

        def gate_prefuse(ah2, aw2, s):
            """Combine g = aw * ah2 into a full fp16 tile during DVE idle
            (load-bound) windows; the mul phase then needs ONE pass."""
            g = gpool.tile([128, 2 * HW], f16, tag=f"g{s}")
            aa = ah2[:, :, :, :]
            a = aw2[:, :, :]
            for cb in range(2):
                gout = bass.AP(tensor=g.tensor, offset=g[:, cb * HW:].offset,
                               ap=[g[:, :].ap[0], [W, H], [2, W // 2], [1, 2]])
                gin0 = bass.AP(tensor=a.tensor, offset=aw2[:, cb, :].offset,
                               ap=[a.ap[0], [0, H], [2, W // 2], [1, 2]])
                gin1 = bass.AP(tensor=aa.tensor, offset=ah2[:, cb].offset,
                               ap=[aa.ap[0], [2, H], [0, W // 2], [1, 2]])
                red.tensor_mul(out=gout, in0=gin0, in1=gin1)
            return g

        def muls_fused_store(s, xt, g):
            """out = x * g in one DVE pass, then store."""
            red.tensor_mul(out=xt[:, :], in0=xt[:, :], in1=g[:, :])
            ost = out[s].rearrange("(cb c) h w -> c cb (h w)", cb=2)
            nc.sync.dma_start(out=ost[:, 0], in_=xt[:, 0:HW])
            nc.scalar.dma_start(out=ost[:, 1], in_=xt[:, HW:2 * HW])

        def muls_and_store(s, xt, ah2, aw2):
            xv = xt.rearrange("p (cb h w) -> p cb h w", cb=2, w=W)
            a = aw2[:, :, :]
            awb = bass.AP(tensor=a.tensor, offset=a.offset,
                          ap=[a.ap[0], a.ap[1], [0, H], a.ap[2]])
            aa = ah2[:, :, :, :]
            ahf = bass.AP(tensor=aa.tensor, offset=aa.offset,
                          ap=[aa.ap[0], [2, 128], [0, W // 2], [1, 2]])
            xf = bass.AP(tensor=xt.tensor, offset=xt[:, :].offset,
                         ap=[xt[:, :].ap[0], [W, 2 * H], [2, W // 2], [1, 2]])
            ost = out[s].rearrange("(cb c) h w -> c cb (h w)", cb=2)
            if s < S - 1:
                red.tensor_mul(out=xv, in0=xv, in1=awb)
                red.tensor_mul(out=xf, in0=xf, in1=ahf)
                nc.sync.dma_start(out=ost[:, 0], in_=xt[:, 0:HW])
                nc.scalar.dma_start(out=ost[:, 1], in_=xt[:, HW:2 * HW])
            else:
                # last sample: per-(cb, h-half) muls with eighth-tile stores
                for cb in range(2):
                    awc = bass.AP(tensor=a.tensor, offset=aw2[:, cb, :].offset,
                                  ap=[a.ap[0], [0, H // 2], a.ap[2]])
                    for hh in range(2):
                        off = cb * HW + hh * (HW // 2)
                        xvc = xt[:, off:off + HW // 2].rearrange(
                            "p (h w) -> p h w", w=W)
                        red.tensor_mul(out=xvc, in0=xvc, in1=awc)
                        xfc = bass.AP(tensor=xt.tensor,
                                      offset=xt[:, off:].offset,
                                      ap=[xt[:, :].ap[0], [W, H // 2],
                                          [2, W // 2], [1, 2]])
                        ahc = bass.AP(tensor=aa.tensor,
                                      offset=ah2[:, cb, hh * (H // 2):].offset,
                                      ap=[aa.ap[0], [2, H // 2],
                                          [0, W // 2], [1, 2]])
                        red.tensor_mul(out=xfc, in0=xfc, in1=ahc)
                        nq = 4 if (cb == 1 and hh == 1) else 2
                        for qq in range(nq):
                            eng = nc.sync if qq % 2 == 0 else nc.scalar
                            csz = (HW // 2) // nq
                            osl = slice(hh * (HW // 2) + qq * csz,
                                        hh * (HW // 2) + (qq + 1) * csz)
                            eng.dma_start(
                                out=ost[:, cb, osl],
                                in_=xt[:, off + qq * csz:off + (qq + 1) * csz])

        # ---------- schedule: trees in sample order, then muls ----------
        y_tiles = []
        for s in range(S):
            yt = ypool.tile([128, 4, T], f16, tag=f"y{s}")
            y_tiles.append(yt)

        gates_out = [None] * S
        # PE col phases hoisted ahead of conv/gates so the PE never waits
        # on a sample's DVE round-trip before starting the next col block.
        row_tree_half(all_xts[0], y_tiles[0], 0, 0)
        row_tree_half(all_xts[0], y_tiles[0], 0, 1)
        c3_0 = col_pe_mm(all_xts[0])
        row_tree_cb(all_xts[0], y_tiles[0], 1)
        c3_1 = col_pe_mm(all_xts[1])
        col_tail(c3_0, y_tiles[0])
        gates_out[0] = gates(y_tiles[0])
        c3_2 = col_pe_mm(all_xts[2])
        g0 = gate_prefuse(*gates_out[0], 0)
        row_tree_joint(all_xts[1], y_tiles[1])
        col_tail(c3_1, y_tiles[1])
        gates_out[1] = gates(y_tiles[1])
        c3_3 = col_pe_mm(all_xts[3])
        g1 = gate_prefuse(*gates_out[1], 1)
        row_tree_joint(all_xts[2], y_tiles[2])
        col_tail(c3_2, y_tiles[2])
        gates_out[2] = gates(y_tiles[2])
        muls_fused_store(0, all_xts[0], g0)
        row_tree_joint(all_xts[3], y_tiles[3])
        col_tail(c3_3, y_tiles[3])
        gates_out[3] = gates(y_tiles[3])
        muls_fused_store(1, all_xts[1], g1)
        muls_and_store(2, all_xts[2], *gates_out[2])
        muls_and_store(3, all_xts[3], *gates_out[3])

    if legalize:
        import concourse.mybir as mybir
        _legalize_waits(nc, mybir)
    return nc


def _prep_params(conv1_w, conv1_b, bn_gamma, bn_beta, bn_mean, bn_var,
                 gate_w, gate_b, convh_w, convh_b, convw_w, convw_b):
    f32 = np.float32
    bnscale = (np.asarray(bn_gamma, f32)
               / np.sqrt(np.asarray(bn_var, f32) + BN_EPS)).astype(f32)
    Wc = np.asarray(conv1_w, f32)[:, :, :, 1]
    s_ci = np.where(np.arange(3 * C) < C, 1.0 / W, 2.0 / W).astype(f32)
    W2 = (Wc * s_ci[None, :, None] * bnscale[:, None, None]).astype(f32)
    bias2 = ((np.asarray(conv1_b, f32) - np.asarray(bn_mean, f32)) * bnscale
             + np.asarray(bn_beta, f32)).astype(f32)
    W6 = W2.reshape(MIP, 6, 128, 3)
    W4 = np.stack([W6[:, 2 + gp] + W6[:, gp % 2] for gp in range(4)], axis=1)
    w2 = np.ascontiguousarray(W4.transpose(2, 1, 3, 0)).astype(np.float16)
    gw = np.full((MIP, 1), float(gate_w) / T, f32)
    gb = np.full((MIP, 1), float(gate_b), f32)
    wh = np.ascontiguousarray(np.asarray(convh_w, np.float16).T)
    ww = np.ascontiguousarray(np.asarray(convw_w, np.float16).T)
    bh = np.ascontiguousarray(np.asarray(convh_b, f32).reshape(2, 128).T)
    bw = np.ascontiguousarray(np.asarray(convw_b, f32).reshape(2, 128).T)
    idn = np.ascontiguousarray(np.eye(128, dtype=np.float16))
    return dict(w2=w2, bias2=bias2.reshape(MIP, 1), gwv=gw, gbv=gb,
                wh=wh, ww=ww, bh=bh, bw=bw, idn=idn)


def kernel(**inputs):
    import sys
    if "/opt/trn_rl_repo" not in sys.path:
        sys.path.insert(0, "/opt/trn_rl_repo")
    from concourse.bass_utils import run_bass_kernel_spmd

    x = np.asarray(inputs["x"], np.float32).astype(np.float16)
    params = _prep_params(
        inputs["conv1_w"], inputs["conv1_b"], inputs["bn_gamma"],
        inputs["bn_beta"], inputs["bn_mean"], inputs["bn_var"],
        inputs["gate_w"], inputs["gate_b"], inputs["convh_w"],
        inputs["convh_b"], inputs["convw_w"], inputs["convw_b"])

    if "nc" not in _CACHE:
        _CACHE["nc"] = _build_program()
    nc = _CACHE["nc"]

    in_maps = [
        {"xs": np.ascontiguousarray(x[i * S:(i + 1) * S]), **params}
        for i in range(N_CORES)
    ]
    res = run_bass_kernel_spmd(nc, in_maps, core_ids=list(range(N_CORES)))
    out = np.concatenate([r["out"] for r in res.results], axis=0)
    return out.astype(np.float32)


# revision 9
# speedup vs baseline: 1.0419x; 1.0223x over previous
"""Trainium2 Bass kernel for nn_EnhancedCoordAtt — v3.

vs baseline (97us -> ~90us):
  - h-gate x2-replication instead of full ACT expansion: ACT writes
    ah2[p,cb,h,0:2] (256 elems, ~0.4us instead of 27us); mulB's in1 AP
    [(cb h) stride-2][w_hi 0-stride][w_lo unit-2] keeps the innermost
    dim unit-stride/2-elem, which is all the DVE 2x_1p mode needs
    (HW-verified 4426ns for [128,8192], same as the full expansion).
  - col-pooling levels L1-L3 (8-row partial sums) moved to the PE as
    identity-lhsT accumulated matmuls into PSUM (16 mm of f=512 per
    sample, contiguous-64 rhs inner dim, fp32-exact), ACT evacuates and
    casts to fp16; DVE only does the last two levels (-3.3us/sample).
    The row tree stays on DVE: its reduction axis IS the contiguous w,
    so PE rhs slices would have 8-elem strided runs (measured 2x cost).
  - PE col phases hoisted ahead of conv/gate matmuls in program order so
    the PE never waits on a sample's DVE round-trip before starting the
    next col block (this PE drag taxed all concurrent DVE muls ~18%).
  - loads/stores striped over both HWDGE rings (single-ring DMA was
    measured to slow concurrent DVE ops by ~18%).
  - s0 trees chunked (cb0 h-halves first) for earliest start; last
    sample's muls/stores chunked per (cb, h-half) for a short tail.
  - gate prefuse for s0/s1: g = aw*ah2 is combined during the DVE's
    load-bound idle windows (~27-33us), so their gate application in the
    packed phase is a single x*g pass instead of two.  (Prefusing the
    later samples was measured neutral-to-worse: their gates only become
    ready mid mul-phase, so the prefuse displaces muls instead of
    filling idle time.)
All shapes hardcoded to the nn_EnhancedCoordAtt_78855599555233 spec.
"""

import numpy as np

N, C, H, W = 32, 256, 64, 64
MIP = 8
N_CORES = 8
S = N // N_CORES
HW = H * W
T = H + W
BN_EPS = 1e-5

_CACHE = {}


def _legalize_waits(nc, mybir, max_keep=1):
    f = nc.m.functions[0]
    for blk in f.blocks:
        out, changed = [], False
        for inst in blk.instructions:
            si = inst.sync_info
            t = type(inst).__name__
            if (si is not None and len(si.on_wait) > max_keep
                    and t != "InstEventSemaphore"):
                waits = list(si.on_wait)
                for j, w in enumerate(waits[:-max_keep]):
                    ev = mybir.InstEventSemaphore(
                        name=f"{inst.name}_xw{j}", ins=[], outs=[])
                    ev.engine = inst.engine
                    ev.sync_info = mybir.SyncInfo(on_wait=[w], on_update=[])
                    out.append(ev)
                inst.sync_info = mybir.SyncInfo(
                    on_wait=waits[-max_keep:], on_update=list(si.on_update))
                changed = True
            out.append(inst)
        if changed:
            blk.instructions = out


def _build_program(legalize=True, sim_compat=False):
    import concourse.bass as bass
    import concourse.tile as tile
    import concourse.mybir as mybir
    from contextlib import ExitStack

    f16 = mybir.dt.float16
    f32 = mybir.dt.float32
    i32 = mybir.dt.int32
    nc = bass.Bass()

    xs = nc.declare_dram_parameter("xs", [S, C, H, W], f16, isOutput=False)
    w2 = nc.declare_dram_parameter("w2", [128, 4, 3, MIP], f16, isOutput=False)
    bias2 = nc.declare_dram_parameter("bias2", [MIP, 1], f32, isOutput=False)
    gwv = nc.declare_dram_parameter("gwv", [MIP, 1], f32, isOutput=False)
    gbv = nc.declare_dram_parameter("gbv", [MIP, 1], f32, isOutput=False)
    wh = nc.declare_dram_parameter("wh", [MIP, C], f16, isOutput=False)
    ww = nc.declare_dram_parameter("ww", [MIP, C], f16, isOutput=False)
    bh = nc.declare_dram_parameter("bh", [128, 2], f32, isOutput=False)
    bw = nc.declare_dram_parameter("bw", [128, 2], f32, isOutput=False)
    idn = nc.declare_dram_parameter("idn", [128, 128], f16, isOutput=False)
    out = nc.declare_dram_parameter("out", [S, C, H, W], f16, isOutput=True)

    with tile.TileContext(nc) as tc, ExitStack() as ctx:
        ctx.enter_context(nc.allow_low_precision(reason="2e-2 tolerance, fp16 path"))
        red = nc.vector
        Sig = mybir.ActivationFunctionType.Sigmoid
        Copy = mybir.ActivationFunctionType.Copy

        singles = ctx.enter_context(tc.tile_pool(name="singles", bufs=1))
        xpool = ctx.enter_context(tc.tile_pool(name="xin", bufs=4))
        ypool = ctx.enter_context(tc.tile_pool(name="yall", bufs=4))
        small = ctx.enter_context(tc.tile_pool(name="small", bufs=4))
        tpool = ctx.enter_context(tc.tile_pool(name="tree", bufs=2))
        cpool = ctx.enter_context(tc.tile_pool(name="colp", bufs=4))
        apool = ctx.enter_context(tc.tile_pool(name="attn", bufs=4))
        gpool = ctx.enter_context(tc.tile_pool(name="gfuse", bufs=1))
        pspool = ctx.enter_context(tc.tile_pool(name="ps", bufs=2, space="PSUM"))
        psgate = ctx.enter_context(tc.tile_pool(name="psg", bufs=1, space="PSUM"))
        pscol = ctx.enter_context(tc.tile_pool(name="psc", bufs=4, space="PSUM"))

        # ---- identity matrix for PE copy/accumulate matmuls (host param) ----
        ident = singles.tile([128, 128], f16)
        nc.gpsimd.dma_start(out=ident, in_=idn[:, :])

        # ---- params on gpsimd SWDGE (never queues behind x traffic) ----
        w2sb = singles.tile([128, 4, 3, MIP], f16)
        nc.gpsimd.dma_start(out=w2sb, in_=w2[:, :, :, :])
        bias2sb = singles.tile([MIP, 1], f32)
        nc.gpsimd.dma_start(out=bias2sb, in_=bias2[:, :])
        gwsb = singles.tile([MIP, 1], f32)
        nc.gpsimd.dma_start(out=gwsb, in_=gwv[:, :])
        gbsb = singles.tile([MIP, 1], f32)
        nc.gpsimd.dma_start(out=gbsb, in_=gbv[:, :])
        whsb = singles.tile([MIP, C], f16)
        nc.gpsimd.dma_start(out=whsb, in_=wh[:, :])
        wwsb = singles.tile([MIP, C], f16)
        nc.gpsimd.dma_start(out=wwsb, in_=ww[:, :])
        bhsb = singles.tile([128, 2], f32)
        nc.gpsimd.dma_start(out=bhsb, in_=bh[:, :])
        bwsb = singles.tile([128, 2], f32)
        nc.gpsimd.dma_start(out=bwsb, in_=bw[:, :])

        # ---- x loads, striped across both rings ----
        all_xts = []
        for s in range(S):
            xt = xpool.tile([128, 2 * HW], f16, tag="xt")
            all_xts.append(xt)
            src_full = xs[s].rearrange("(cb c) h w -> c cb (h w)", cb=2)
            if s <= 1:
                # per-(cb, h-half) chunks on both rings: finer arrival
                # granularity lets s0/s1 tree work start earlier
                for cb in range(2):
                    for hh in range(2):
                        eng = nc.sync if hh == 0 else nc.scalar
                        sl = slice(cb * HW + hh * (HW // 2),
                                   cb * HW + (hh + 1) * (HW // 2))
                        eng.dma_start(
                            out=xt[:, sl],
                            in_=src_full[:, cb, hh * (HW // 2):(hh + 1) * (HW // 2)])
            else:
                for cb in range(2):
                    eng = nc.sync if cb == 0 else nc.scalar
                    eng.dma_start(
                        out=xt[:, cb * HW:(cb + 1) * HW], in_=src_full[:, cb])

        def row_tree_half(xt, y_all, cb, i):
            """Row-half sums for one (cb, h-half) chunk (earliest start)."""
            xh = xt[:, cb * HW + i * (HW // 2): cb * HW + (i + 1) * (HW // 2)]
            rv = xh.rearrange("p (hj w) -> p hj w", w=W // 2)
            r1 = tpool.tile([128, H, 16], f16, tag="hr1")
            red.tensor_add(out=r1, in0=rv[:, :, 0:16], in1=rv[:, :, 16:32])
            r2 = tpool.tile([128, H, 8], f16, tag="hr2")
            red.tensor_add(out=r2, in0=r1[:, :, 0:8], in1=r1[:, :, 8:16])
            r3 = tpool.tile([128, H, 4], f16, tag="hr3")
            red.tensor_add(out=r3, in0=r2[:, :, 0:4], in1=r2[:, :, 4:8])
            r4 = tpool.tile([128, H, 2], f16, tag="hr4")
            red.tensor_add(out=r4, in0=r3[:, :, 0:2], in1=r3[:, :, 2:4])
            rv4 = r4.rearrange("p (h j) a -> p j h a", j=2)
            red.tensor_add(
                out=y_all.rearrange("p (j c) t -> p c j t", j=2)[
                    :, cb, :, i * (H // 2):(i + 1) * (H // 2)],
                in0=rv4[:, :, :, 0], in1=rv4[:, :, :, 1])

        def row_tree_cb(xt, y_all, cb):
            """Row-half sums for one cb block (s0 path: starts on cb0)."""
            xc = xt[:, cb * HW:(cb + 1) * HW]
            rv = xc.rearrange("p (hj w) -> p hj w", w=W // 2)
            r1 = tpool.tile([128, 2 * H, 16], f16, tag="r1")
            red.tensor_add(out=r1, in0=rv[:, :, 0:16], in1=rv[:, :, 16:32])
            r2 = tpool.tile([128, 2 * H, 8], f16, tag="r2")
            red.tensor_add(out=r2, in0=r1[:, :, 0:8], in1=r1[:, :, 8:16])
            r3 = tpool.tile([128, 2 * H, 4], f16, tag="r3")
            red.tensor_add(out=r3, in0=r2[:, :, 0:4], in1=r2[:, :, 4:8])
            r4 = tpool.tile([128, 2 * H, 2], f16, tag="r4")
            red.tensor_add(out=r4, in0=r3[:, :, 0:2], in1=r3[:, :, 2:4])
            rv4 = r4.rearrange("p (h j) a -> p j h a", j=2)
            red.tensor_add(
                out=y_all.rearrange("p (j c) t -> p c j t", j=2)[:, cb, :, 0:H],
                in0=rv4[:, :, :, 0], in1=rv4[:, :, :, 1])

        def row_tree_joint(xt, y_all):
            rv = xt.rearrange("p (cb hj w) -> p cb hj w", cb=2, w=W // 2)
            r1 = tpool.tile([128, 2, 2 * H, 16], f16, tag="jr1")
            red.tensor_add(out=r1, in0=rv[:, :, :, 0:16], in1=rv[:, :, :, 16:32])
            r2 = tpool.tile([128, 2, 2 * H, 8], f16, tag="jr2")
            red.tensor_add(out=r2, in0=r1[:, :, :, 0:8], in1=r1[:, :, :, 8:16])
            r3 = tpool.tile([128, 2, 2 * H, 4], f16, tag="jr3")
            red.tensor_add(out=r3, in0=r2[:, :, :, 0:4], in1=r2[:, :, :, 4:8])
            r4 = tpool.tile([128, 2, 2 * H, 2], f16, tag="jr4")
            red.tensor_add(out=r4, in0=r3[:, :, :, 0:2], in1=r3[:, :, :, 2:4])
            rv4 = r4.rearrange("p cb (h j) a -> p cb j h a", j=2)
            red.tensor_add(
                out=y_all.rearrange("p (j c) t -> p c j t", j=2)[:, :, :, 0:H],
                in0=rv4[:, :, :, :, 0], in1=rv4[:, :, :, :, 1])

        def col_pe_mm(xt):
            """Col L1-L3 (8-row partials) on PE + ACT evac/cast, per cb."""
            c3es = []
            for cb in range(2):
                psc = pscol.tile([128, 8, W], f32, tag="psc")
                for r in range(8):
                    base = xt[:, cb * HW + r * W:]
                    rhs = bass.AP(tensor=base.tensor, offset=base.offset,
                                  ap=[base.ap[0], [8 * W, 8], [1, W]])
                    nc.tensor.matmul(out=psc, lhsT=ident[:, :], rhs=rhs,
                                     start=(r == 0), stop=(r == 7))
                c3e = cpool.tile([128, 8, W], f16, tag="c3e")
                nc.scalar.activation(out=c3e, in_=psc, func=Copy,
                                     bias=0.0, scale=1.0)
                c3es.append(c3e)
            return c3es

        def col_tail(c3es, y_all):
            """Last two col levels on DVE."""
            for cb in range(2):
                c3e = c3es[cb]
                c4 = cpool.tile([128, 2, 2, W], f16, tag="c4")
                cv = c3e.rearrange("p (i m) w -> p i m w", i=2)
                red.tensor_add(out=c4, in0=cv[:, :, 0:2], in1=cv[:, :, 2:4])
                red.tensor_add(
                    out=y_all.rearrange("p (i c) t -> p i c t", i=2)[:, :, cb, H:T],
                    in0=c4[:, :, 0, :], in1=c4[:, :, 1, :])

        def gates(y_all):
            psy = pspool.tile([MIP, T], f32, tag="psy")
            order = [(0, 1)] + [(g, k) for g in range(4) for k in range(3)
                                if (g, k) != (0, 1)]
            for idx, (g, k) in enumerate(order):
                lhsT = w2sb[:, g, k, :]
                if k == 1:
                    o_sl, i_sl = slice(0, T), slice(0, T)
                elif k == 0:
                    o_sl, i_sl = slice(2, T), slice(0, T - 2)
                else:
                    o_sl, i_sl = slice(0, T - 2), slice(2, T)
                nc.tensor.matmul(
                    out=psy[:, o_sl], lhsT=lhsT, rhs=y_all[:, g, i_sl],
                    start=(idx == 0), stop=(idx == len(order) - 1))

            ya0 = small.tile([MIP, T], f32, tag="ya0")
            nc.vector.tensor_scalar_add(out=ya0, in0=psy, scalar1=bias2sb[:, :])
            ysg = small.tile([MIP, T], f32, tag="ysg")
            nc.scalar.activation(out=ysg, in_=ya0, func=Sig, bias=0.0, scale=1.0)
            ya = small.tile([MIP, T], f32, tag="ya")
            red.tensor_mul(out=ya, in0=ya0, in1=ysg)
            ysum = small.tile([MIP, 1], f32, tag="ysum")
            red.reduce_sum(out=ysum, in_=ya, axis=mybir.AxisListType.X)
            se = small.tile([MIP, 1], f32, tag="se")
            nc.scalar.activation(out=se, in_=ysum, func=Sig,
                                 bias=gbsb[:, :], scale=gwsb[:, :])
            yg = small.tile([MIP, T], f16, tag="yg")
            nc.vector.tensor_scalar_mul(out=yg, in0=ya, scalar1=se[:, :])

            ah2 = apool.tile([128, 2, H, 2], f16, tag="ah2")
            aw2 = apool.tile([128, 2, W], f16, tag="aw2")
            for cb in range(2):
                psa = psgate.tile([128, H], f32, tag="psa")
                nc.tensor.matmul(
                    out=psa, lhsT=whsb[:, cb * 128:(cb + 1) * 128],
                    rhs=yg[:, 0:H], start=True, stop=True)
                pa = psa[:, :]
                pab = bass.AP(tensor=pa.tensor, offset=pa.offset,
                              ap=[pa.ap[0], pa.ap[1], [0, 2]])
                nc.scalar.activation(out=ah2[:, cb], in_=pab, func=Sig,
                                     bias=bhsb[:, cb:cb + 1], scale=1.0)
                psb = psgate.tile([128, W], f32, tag="psb")
                nc.tensor.matmul(
                    out=psb, lhsT=wwsb[:, cb * 128:(cb + 1) * 128],
                    rhs=yg[:, H:T], start=True, stop=True)
                nc.scalar.activation(out=aw2[:, cb], in_=psb, func=Sig,
                                     bias=bwsb[:, cb:cb + 1], scale=1.0)
            return ah2, aw2

        def gate_prefuse(ah2, aw2, s):
            """Combine g = aw * ah2 into a full fp16 tile during DVE idle
            (load-bound) windows; the mul phase then needs ONE pass."""
            g = gpool.tile([128, 2 * HW], f16, tag=f"g{s}")
            aa = ah2[:, :, :, :]
            a = aw2[:, :, :]
            for cb in range(2):
                gout = bass.AP(tensor=g.tensor, offset=g[:, cb * HW:].offset,
                               ap=[g[:, :].ap[0], [W, H], [2, W // 2], [1, 2]])
                gin0 = bass.AP(tensor=a.tensor, offset=aw2[:, cb, :].offset,
                               ap=[a.ap[0], [0, H], [2, W // 2], [1, 2]])
                gin1 = bass.AP(tensor=aa.tensor, offset=ah2[:, cb].offset,
                               ap=[aa.ap[0], [2, H], [0, W // 2], [1, 2]])
                red.tensor_mul(out=gout, in0=gin0, in1=gin1)
            return g

        def muls_fused_store(s, xt, g):
            """out = x * g in one DVE pass, then store."""
            red.tensor_mul(out=xt[:, :], in0=xt[:, :], in1=g[:, :])
            ost = out[s].rearrange("(cb c) h w -> c cb (h w)", cb=2)
            nc.sync.dma_start(out=ost[:, 0], in_=xt[:, 0:HW])
            nc.scalar.dma_start(out=ost[:, 1], in_=xt[:, HW:2 * HW])

        def muls_and_store(s, xt, ah2, aw2):
            xv = xt.rearrange("p (cb h w) -> p cb h w", cb=2, w=W)
            a = aw2[:, :, :]
            awb = bass.AP(tensor=a.tensor, offset=a.offset,
                          ap=[a.ap[0], a.ap[1], [0, H], a.ap[2]])
            aa = ah2[:, :, :, :]
            ahf = bass.AP(tensor=aa.tensor, offset=aa.offset,
                          ap=[aa.ap[0], [2, 128], [0, W // 2], [1, 2]])
            xf = bass.AP(tensor=xt.tensor, offset=xt[:, :].offset,
                         ap=[xt[:, :].ap[0], [W, 2 * H], [2, W // 2], [1, 2]])
            ost = out[s].rearrange("(cb c) h w -> c cb (h w)", cb=2)
            if s < S - 1:
                red.tensor_mul(out=xv, in0=xv, in1=awb)
                red.tensor_mul(out=xf, in0=xf, in1=ahf)
                nc.sync.dma_start(out=ost[:, 0], in_=xt[:, 0:HW])
                nc.scalar.dma_start(out=ost[:, 1], in_=xt[:, HW:2 * HW])
            else:
                # last sample: per-(cb, h-half) muls with eighth-tile stores
                for cb in range(2):
                    awc = bass.AP(tensor=a.tensor, offset=aw2[:, cb, :].offset,
                                  ap=[a.ap[0], [0, H // 2], a.ap[2]])
                    for hh in range(2):
                        off = cb * HW + hh * (HW // 2)
                        xvc = xt[:, off:off + HW // 2].rearrange(
                            "p (h w) -> p h w", w=W)
                        red.tensor_mul(out=xvc, in0=xvc, in1=awc)
                        xfc = bass.AP(tensor=xt.tensor,
                                      offset=xt[:, off:].offset,
                                      ap=[xt[:, :].ap[0], [W, H // 2],
                                          [2, W // 2], [1, 2]])
                        ahc = bass.AP(tensor=aa.tensor,
                                      offset=ah2[:, cb, hh * (H // 2):].offset,
                                      ap=[aa.ap[0], [2, H // 2],
                                          [0, W // 2], [1, 2]])
                        red.tensor_mul(out=xfc, in0=xfc, in1=ahc)
                        nq = 4 if (cb == 1 and hh == 1) else 2
                        for qq in range(nq):
                            eng = nc.sync if qq % 2 == 0 else nc.scalar
                            csz = (HW // 2) // nq
                            osl = slice(hh * (HW // 2) + qq * csz,
                                        hh * (HW // 2) + (qq + 1) * csz)
                            eng.dma_start(
                                out=ost[:, cb, osl],
                                in_=xt[:, off + qq * csz:off + (qq + 1) * csz])

        # ---------- schedule: trees in sample order, then muls ----------
        y_tiles = []
        for s in range(S):
            yt = ypool.tile([128, 4, T], f16, tag=f"y{s}")
            y_tiles.append(yt)

        gates_out = [None] * S
        # PE col phases hoisted ahead of conv/gates so the PE never waits
        # on a sample's DVE round-trip before starting the next col block.
        row_tree_half(all_xts[0], y_tiles[0], 0, 0)
        row_tree_half(all_xts[0], y_tiles[0], 0, 1)
        c3_0 = col_pe_mm(all_xts[0])
        row_tree_cb(all_xts[0], y_tiles[0], 1)
        c3_1 = col_pe_mm(all_xts[1])
        col_tail(c3_0, y_tiles[0])
        gates_out[0] = gates(y_tiles[0])
        c3_2 = col_pe_mm(all_xts[2])
        g0 = gate_prefuse(*gates_out[0], 0)
        row_tree_half(all_xts[1], y_tiles[1], 0, 0)
        row_tree_half(all_xts[1], y_tiles[1], 0, 1)
        row_tree_half(all_xts[1], y_tiles[1], 1, 0)
        row_tree_half(all_xts[1], y_tiles[1], 1, 1)
        col_tail(c3_1, y_tiles[1])
        gates_out[1] = gates(y_tiles[1])
        c3_3 = col_pe_mm(all_xts[3])
        g1 = gate_prefuse(*gates_out[1], 1)
        row_tree_joint(all_xts[2], y_tiles[2])
        col_tail(c3_2, y_tiles[2])
        gates_out[2] = gates(y_tiles[2])
        muls_fused_store(0, all_xts[0], g0)
        row_tree_joint(all_xts[3], y_tiles[3])
        col_tail(c3_3, y_tiles[3])
        gates_out[3] = gates(y_tiles[3])
        muls_fused_store(1, all_xts[1], g1)
        muls_and_store(2, all_xts[2], *gates_out[2])
        muls_and_store(3, all_xts[3], *gates_out[3])

    if legalize:
        import concourse.mybir as mybir
        _legalize_waits(nc, mybir)
    return nc


def _prep_params(conv1_w, conv1_b, bn_gamma, bn_beta, bn_mean, bn_var,
                 gate_w, gate_b, convh_w, convh_b, convw_w, convw_b):
    f32 = np.float32
    bnscale = (np.asarray(bn_gamma, f32)
               / np.sqrt(np.asarray(bn_var, f32) + BN_EPS)).astype(f32)
    Wc = np.asarray(conv1_w, f32)[:, :, :, 1]
    s_ci = np.where(np.arange(3 * C) < C, 1.0 / W, 2.0 / W).astype(f32)
    W2 = (Wc * s_ci[None, :, None] * bnscale[:, None, None]).astype(f32)
    bias2 = ((np.asarray(conv1_b, f32) - np.asarray(bn_mean, f32)) * bnscale
             + np.asarray(bn_beta, f32)).astype(f32)
    W6 = W2.reshape(MIP, 6, 128, 3)
    W4 = np.stack([W6[:, 2 + gp] + W6[:, gp % 2] for gp in range(4)], axis=1)
    w2 = np.ascontiguousarray(W4.transpose(2, 1, 3, 0)).astype(np.float16)
    gw = np.full((MIP, 1), float(gate_w) / T, f32)
    gb = np.full((MIP, 1), float(gate_b), f32)
    wh = np.ascontiguousarray(np.asarray(convh_w, np.float16).T)
    ww = np.ascontiguousarray(np.asarray(convw_w, np.float16).T)
    bh = np.ascontiguousarray(np.asarray(convh_b, f32).reshape(2, 128).T)
    bw = np.ascontiguousarray(np.asarray(convw_b, f32).reshape(2, 128).T)
    idn = np.ascontiguousarray(np.eye(128, dtype=np.float16))
    return dict(w2=w2, bias2=bias2.reshape(MIP, 1), gwv=gw, gbv=gb,
                wh=wh, ww=ww, bh=bh, bw=bw, idn=idn)


def kernel(**inputs):
    import sys
    if "/opt/trn_rl_repo" not in sys.path:
        sys.path.insert(0, "/opt/trn_rl_repo")
    from concourse.bass_utils import run_bass_kernel_spmd

    x = np.asarray(inputs["x"], np.float32).astype(np.float16)
    params = _prep_params(
        inputs["conv1_w"], inputs["conv1_b"], inputs["bn_gamma"],
        inputs["bn_beta"], inputs["bn_mean"], inputs["bn_var"],
        inputs["gate_w"], inputs["gate_b"], inputs["convh_w"],
        inputs["convh_b"], inputs["convw_w"], inputs["convw_b"])

    if "nc" not in _CACHE:
        _CACHE["nc"] = _build_program()
    nc = _CACHE["nc"]

    in_maps = [
        {"xs": np.ascontiguousarray(x[i * S:(i + 1) * S]), **params}
        for i in range(N_CORES)
    ]
    res = run_bass_kernel_spmd(nc, in_maps, core_ids=list(range(N_CORES)))
    out = np.concatenate([r["out"] for r in res.results], axis=0)
    return out.astype(np.float32)


# revision 10
# speedup vs baseline: 1.0556x; 1.0132x over previous
"""Trainium2 Bass kernel for nn_EnhancedCoordAtt — v3.

vs baseline (97us -> ~90us):
  - h-gate x2-replication instead of full ACT expansion: ACT writes
    ah2[p,cb,h,0:2] (256 elems, ~0.4us instead of 27us); mulB's in1 AP
    [(cb h) stride-2][w_hi 0-stride][w_lo unit-2] keeps the innermost
    dim unit-stride/2-elem, which is all the DVE 2x_1p mode needs
    (HW-verified 4426ns for [128,8192], same as the full expansion).
  - col-pooling levels L1-L3 (8-row partial sums) moved to the PE as
    identity-lhsT accumulated matmuls into PSUM (16 mm of f=512 per
    sample, contiguous-64 rhs inner dim, fp32-exact), ACT evacuates and
    casts to fp16; DVE only does the last two levels (-3.3us/sample).
    The row tree stays on DVE: its reduction axis IS the contiguous w,
    so PE rhs slices would have 8-elem strided runs (measured 2x cost).
  - PE col phases hoisted ahead of conv/gate matmuls in program order so
    the PE never waits on a sample's DVE round-trip before starting the
    next col block (this PE drag taxed all concurrent DVE muls ~18%).
  - loads/stores striped over both HWDGE rings (single-ring DMA was
    measured to slow concurrent DVE ops by ~18%).
  - s0 trees chunked (cb0 h-halves first) for earliest start; last
    sample's muls/stores chunked per (cb, h-half) for a short tail.
  - gate prefuse for s0/s1: g = aw*ah2 is combined during the DVE's
    load-bound idle windows (~27-33us), so their gate application in the
    packed phase is a single x*g pass instead of two.  (Prefusing the
    later samples was measured neutral-to-worse: their gates only become
    ready mid mul-phase, so the prefuse displaces muls instead of
    filling idle time.)
All shapes hardcoded to the nn_EnhancedCoordAtt_78855599555233 spec.
"""

import numpy as np

N, C, H, W = 32, 256, 64, 64
MIP = 8
N_CORES = 8
S = N // N_CORES
HW = H * W
T = H + W
BN_EPS = 1e-5

_CACHE = {}


def _legalize_waits(nc, mybir, max_keep=1):
    f = nc.m.functions[0]
    for blk in f.blocks:
        out, changed = [], False
        for inst in blk.instructions:
            si = inst.sync_info
            t = type(inst).__name__
            if (si is not None and len(si.on_wait) > max_keep
                    and t != "InstEventSemaphore"):
                waits = list(si.on_wait)
                for j, w in enumerate(waits[:-max_keep]):
                    ev = mybir.InstEventSemaphore(
                        name=f"{inst.name}_xw{j}", ins=[], outs=[])
                    ev.engine = inst.engine
                    ev.sync_info = mybir.SyncInfo(on_wait=[w], on_update=[])
                    out.append(ev)
                inst.sync_info = mybir.SyncInfo(
                    on_wait=waits[-max_keep:], on_update=list(si.on_update))
                changed = True
            out.append(inst)
        if changed:
            blk.instructions = out


def _build_program(legalize=True, sim_compat=False):
    import concourse.bass as bass
    import concourse.tile as tile
    import concourse.mybir as mybir
    from contextlib import ExitStack

    f16 = mybir.dt.float16
    f32 = mybir.dt.float32
    i32 = mybir.dt.int32
    nc = bass.Bass()

    xs = nc.declare_dram_parameter("xs", [S, C, H, W], f16, isOutput=False)
    w2 = nc.declare_dram_parameter("w2", [128, 4, 3, MIP], f16, isOutput=False)
    bias2 = nc.declare_dram_parameter("bias2", [MIP, 1], f32, isOutput=False)
    gwv = nc.declare_dram_parameter("gwv", [MIP, 1], f32, isOutput=False)
    gbv = nc.declare_dram_parameter("gbv", [MIP, 1], f32, isOutput=False)
    wh = nc.declare_dram_parameter("wh", [MIP, C], f16, isOutput=False)
    ww = nc.declare_dram_parameter("ww", [MIP, C], f16, isOutput=False)
    bh = nc.declare_dram_parameter("bh", [128, 2], f32, isOutput=False)
    bw = nc.declare_dram_parameter("bw", [128, 2], f32, isOutput=False)
    idn = nc.declare_dram_parameter("idn", [128, 128], f16, isOutput=False)
    out = nc.declare_dram_parameter("out", [S, C, H, W], f16, isOutput=True)

    with tile.TileContext(nc) as tc, ExitStack() as ctx:
        ctx.enter_context(nc.allow_low_precision(reason="2e-2 tolerance, fp16 path"))
        red = nc.vector
        Sig = mybir.ActivationFunctionType.Sigmoid
        Copy = mybir.ActivationFunctionType.Copy

        singles = ctx.enter_context(tc.tile_pool(name="singles", bufs=1))
        xpool = ctx.enter_context(tc.tile_pool(name="xin", bufs=4))
        ypool = ctx.enter_context(tc.tile_pool(name="yall", bufs=4))
        small = ctx.enter_context(tc.tile_pool(name="small", bufs=4))
        tpool = ctx.enter_context(tc.tile_pool(name="tree", bufs=2))
        cpool = ctx.enter_context(tc.tile_pool(name="colp", bufs=4))
        apool = ctx.enter_context(tc.tile_pool(name="attn", bufs=4))
        gpool = ctx.enter_context(tc.tile_pool(name="gfuse", bufs=1))
        pspool = ctx.enter_context(tc.tile_pool(name="ps", bufs=2, space="PSUM"))
        psgate = ctx.enter_context(tc.tile_pool(name="psg", bufs=1, space="PSUM"))
        pscol = ctx.enter_context(tc.tile_pool(name="psc", bufs=4, space="PSUM"))

        # ---- identity matrix for PE copy/accumulate matmuls (host param) ----
        ident = singles.tile([128, 128], f16)
        nc.gpsimd.dma_start(out=ident, in_=idn[:, :])

        # ---- params on gpsimd SWDGE (never queues behind x traffic) ----
        w2sb = singles.tile([128, 4, 3, MIP], f16)
        nc.gpsimd.dma_start(out=w2sb, in_=w2[:, :, :, :])
        bias2sb = singles.tile([MIP, 1], f32)
        nc.gpsimd.dma_start(out=bias2sb, in_=bias2[:, :])
        gwsb = singles.tile([MIP, 1], f32)
        nc.gpsimd.dma_start(out=gwsb, in_=gwv[:, :])
        gbsb = singles.tile([MIP, 1], f32)
        nc.gpsimd.dma_start(out=gbsb, in_=gbv[:, :])
        whsb = singles.tile([MIP, C], f16)
        nc.gpsimd.dma_start(out=whsb, in_=wh[:, :])
        wwsb = singles.tile([MIP, C], f16)
        nc.gpsimd.dma_start(out=wwsb, in_=ww[:, :])
        bhsb = singles.tile([128, 2], f32)
        nc.gpsimd.dma_start(out=bhsb, in_=bh[:, :])
        bwsb = singles.tile([128, 2], f32)
        nc.gpsimd.dma_start(out=bwsb, in_=bw[:, :])

        # ---- x loads, striped across both rings ----
        all_xts = []
        for s in range(S):
            xt = xpool.tile([128, 2 * HW], f16, tag="xt")
            all_xts.append(xt)
            src_full = xs[s].rearrange("(cb c) h w -> c cb (h w)", cb=2)
            if s <= 2:
                # per-(cb, h-half) chunks on both rings: finer arrival
                # granularity lets early-sample tree work start sooner
                for cb in range(2):
                    for hh in range(2):
                        eng = nc.sync if hh == 0 else nc.scalar
                        sl = slice(cb * HW + hh * (HW // 2),
                                   cb * HW + (hh + 1) * (HW // 2))
                        eng.dma_start(
                            out=xt[:, sl],
                            in_=src_full[:, cb, hh * (HW // 2):(hh + 1) * (HW // 2)])
            else:
                for cb in range(2):
                    eng = nc.sync if cb == 0 else nc.scalar
                    eng.dma_start(
                        out=xt[:, cb * HW:(cb + 1) * HW], in_=src_full[:, cb])

        def row_tree_half(xt, y_all, cb, i):
            """Row-half sums for one (cb, h-half) chunk (earliest start)."""
            xh = xt[:, cb * HW + i * (HW // 2): cb * HW + (i + 1) * (HW // 2)]
            rv = xh.rearrange("p (hj w) -> p hj w", w=W // 2)
            r1 = tpool.tile([128, H, 16], f16, tag="hr1")
            red.tensor_add(out=r1, in0=rv[:, :, 0:16], in1=rv[:, :, 16:32])
            r2 = tpool.tile([128, H, 8], f16, tag="hr2")
            red.tensor_add(out=r2, in0=r1[:, :, 0:8], in1=r1[:, :, 8:16])
            r3 = tpool.tile([128, H, 4], f16, tag="hr3")
            red.tensor_add(out=r3, in0=r2[:, :, 0:4], in1=r2[:, :, 4:8])
            r4 = tpool.tile([128, H, 2], f16, tag="hr4")
            red.tensor_add(out=r4, in0=r3[:, :, 0:2], in1=r3[:, :, 2:4])
            rv4 = r4.rearrange("p (h j) a -> p j h a", j=2)
            red.tensor_add(
                out=y_all.rearrange("p (j c) t -> p c j t", j=2)[
                    :, cb, :, i * (H // 2):(i + 1) * (H // 2)],
                in0=rv4[:, :, :, 0], in1=rv4[:, :, :, 1])

        def row_tree_cb(xt, y_all, cb):
            """Row-half sums for one cb block (s0 path: starts on cb0)."""
            xc = xt[:, cb * HW:(cb + 1) * HW]
            rv = xc.rearrange("p (hj w) -> p hj w", w=W // 2)
            r1 = tpool.tile([128, 2 * H, 16], f16, tag="r1")
            red.tensor_add(out=r1, in0=rv[:, :, 0:16], in1=rv[:, :, 16:32])
            r2 = tpool.tile([128, 2 * H, 8], f16, tag="r2")
            red.tensor_add(out=r2, in0=r1[:, :, 0:8], in1=r1[:, :, 8:16])
            r3 = tpool.tile([128, 2 * H, 4], f16, tag="r3")
            red.tensor_add(out=r3, in0=r2[:, :, 0:4], in1=r2[:, :, 4:8])
            r4 = tpool.tile([128, 2 * H, 2], f16, tag="r4")
            red.tensor_add(out=r4, in0=r3[:, :, 0:2], in1=r3[:, :, 2:4])
            rv4 = r4.rearrange("p (h j) a -> p j h a", j=2)
            red.tensor_add(
                out=y_all.rearrange("p (j c) t -> p c j t", j=2)[:, cb, :, 0:H],
                in0=rv4[:, :, :, 0], in1=rv4[:, :, :, 1])

        def row_tree_joint(xt, y_all):
            rv = xt.rearrange("p (cb hj w) -> p cb hj w", cb=2, w=W // 2)
            r1 = tpool.tile([128, 2, 2 * H, 16], f16, tag="jr1")
            red.tensor_add(out=r1, in0=rv[:, :, :, 0:16], in1=rv[:, :, :, 16:32])
            r2 = tpool.tile([128, 2, 2 * H, 8], f16, tag="jr2")
            red.tensor_add(out=r2, in0=r1[:, :, :, 0:8], in1=r1[:, :, :, 8:16])
            r3 = tpool.tile([128, 2, 2 * H, 4], f16, tag="jr3")
            red.tensor_add(out=r3, in0=r2[:, :, :, 0:4], in1=r2[:, :, :, 4:8])
            r4 = tpool.tile([128, 2, 2 * H, 2], f16, tag="jr4")
            red.tensor_add(out=r4, in0=r3[:, :, :, 0:2], in1=r3[:, :, :, 2:4])
            rv4 = r4.rearrange("p cb (h j) a -> p cb j h a", j=2)
            red.tensor_add(
                out=y_all.rearrange("p (j c) t -> p c j t", j=2)[:, :, :, 0:H],
                in0=rv4[:, :, :, :, 0], in1=rv4[:, :, :, :, 1])

        def col_pe_mm(xt):
            """Col L1-L3 (8-row partials) on PE + ACT evac/cast, per cb."""
            c3es = []
            for cb in range(2):
                psc = pscol.tile([128, 8, W], f32, tag="psc")
                for r in range(8):
                    base = xt[:, cb * HW + r * W:]
                    rhs = bass.AP(tensor=base.tensor, offset=base.offset,
                                  ap=[base.ap[0], [8 * W, 8], [1, W]])
                    nc.tensor.matmul(out=psc, lhsT=ident[:, :], rhs=rhs,
                                     start=(r == 0), stop=(r == 7))
                c3e = cpool.tile([128, 8, W], f16, tag="c3e")
                nc.scalar.activation(out=c3e, in_=psc, func=Copy,
                                     bias=0.0, scale=1.0)
                c3es.append(c3e)
            return c3es

        def col_tail(c3es, y_all):
            """Last two col levels on DVE."""
            for cb in range(2):
                c3e = c3es[cb]
                c4 = cpool.tile([128, 2, 2, W], f16, tag="c4")
                cv = c3e.rearrange("p (i m) w -> p i m w", i=2)
                red.tensor_add(out=c4, in0=cv[:, :, 0:2], in1=cv[:, :, 2:4])
                red.tensor_add(
                    out=y_all.rearrange("p (i c) t -> p i c t", i=2)[:, :, cb, H:T],
                    in0=c4[:, :, 0, :], in1=c4[:, :, 1, :])

        def gates(y_all):
            psy = pspool.tile([MIP, T], f32, tag="psy")
            order = [(0, 1)] + [(g, k) for g in range(4) for k in range(3)
                                if (g, k) != (0, 1)]
            for idx, (g, k) in enumerate(order):
                lhsT = w2sb[:, g, k, :]
                if k == 1:
                    o_sl, i_sl = slice(0, T), slice(0, T)
                elif k == 0:
                    o_sl, i_sl = slice(2, T), slice(0, T - 2)
                else:
                    o_sl, i_sl = slice(0, T - 2), slice(2, T)
                nc.tensor.matmul(
                    out=psy[:, o_sl], lhsT=lhsT, rhs=y_all[:, g, i_sl],
                    start=(idx == 0), stop=(idx == len(order) - 1))

            ya0 = small.tile([MIP, T], f32, tag="ya0")
            nc.vector.tensor_scalar_add(out=ya0, in0=psy, scalar1=bias2sb[:, :])
            ysg = small.tile([MIP, T], f32, tag="ysg")
            nc.scalar.activation(out=ysg, in_=ya0, func=Sig, bias=0.0, scale=1.0)
            ya = small.tile([MIP, T], f32, tag="ya")
            red.tensor_mul(out=ya, in0=ya0, in1=ysg)
            ysum = small.tile([MIP, 1], f32, tag="ysum")
            red.reduce_sum(out=ysum, in_=ya, axis=mybir.AxisListType.X)
            se = small.tile([MIP, 1], f32, tag="se")
            nc.scalar.activation(out=se, in_=ysum, func=Sig,
                                 bias=gbsb[:, :], scale=gwsb[:, :])
            yg = small.tile([MIP, T], f16, tag="yg")
            nc.vector.tensor_scalar_mul(out=yg, in0=ya, scalar1=se[:, :])

            ah2 = apool.tile([128, 2, H, 2], f16, tag="ah2")
            aw2 = apool.tile([128, 2, W], f16, tag="aw2")
            for cb in range(2):
                psa = psgate.tile([128, H], f32, tag="psa")
                nc.tensor.matmul(
                    out=psa, lhsT=whsb[:, cb * 128:(cb + 1) * 128],
                    rhs=yg[:, 0:H], start=True, stop=True)
                pa = psa[:, :]
                pab = bass.AP(tensor=pa.tensor, offset=pa.offset,
                              ap=[pa.ap[0], pa.ap[1], [0, 2]])
                nc.scalar.activation(out=ah2[:, cb], in_=pab, func=Sig,
                                     bias=bhsb[:, cb:cb + 1], scale=1.0)
                psb = psgate.tile([128, W], f32, tag="psb")
                nc.tensor.matmul(
                    out=psb, lhsT=wwsb[:, cb * 128:(cb + 1) * 128],
                    rhs=yg[:, H:T], start=True, stop=True)
                nc.scalar.activation(out=aw2[:, cb], in_=psb, func=Sig,
                                     bias=bwsb[:, cb:cb + 1], scale=1.0)
            return ah2, aw2

        def gate_prefuse(ah2, aw2, s):
            """Combine g = aw * ah2 into a full fp16 tile during DVE idle
            (load-bound) windows; the mul phase then needs ONE pass."""
            g = gpool.tile([128, 2 * HW], f16, tag=f"g{s}")
            aa = ah2[:, :, :, :]
            a = aw2[:, :, :]
            for cb in range(2):
                gout = bass.AP(tensor=g.tensor, offset=g[:, cb * HW:].offset,
                               ap=[g[:, :].ap[0], [W, H], [2, W // 2], [1, 2]])
                gin0 = bass.AP(tensor=a.tensor, offset=aw2[:, cb, :].offset,
                               ap=[a.ap[0], [0, H], [2, W // 2], [1, 2]])
                gin1 = bass.AP(tensor=aa.tensor, offset=ah2[:, cb].offset,
                               ap=[aa.ap[0], [2, H], [0, W // 2], [1, 2]])
                red.tensor_mul(out=gout, in0=gin0, in1=gin1)
            return g

        def muls_fused_store(s, xt, g):
            """out = x * g in one DVE pass, then store."""
            red.tensor_mul(out=xt[:, :], in0=xt[:, :], in1=g[:, :])
            ost = out[s].rearrange("(cb c) h w -> c cb (h w)", cb=2)
            nc.sync.dma_start(out=ost[:, 0], in_=xt[:, 0:HW])
            nc.scalar.dma_start(out=ost[:, 1], in_=xt[:, HW:2 * HW])

        def muls_and_store(s, xt, ah2, aw2):
            xv = xt.rearrange("p (cb h w) -> p cb h w", cb=2, w=W)
            a = aw2[:, :, :]
            awb = bass.AP(tensor=a.tensor, offset=a.offset,
                          ap=[a.ap[0], a.ap[1], [0, H], a.ap[2]])
            aa = ah2[:, :, :, :]
            ahf = bass.AP(tensor=aa.tensor, offset=aa.offset,
                          ap=[aa.ap[0], [2, 128], [0, W // 2], [1, 2]])
            xf = bass.AP(tensor=xt.tensor, offset=xt[:, :].offset,
                         ap=[xt[:, :].ap[0], [W, 2 * H], [2, W // 2], [1, 2]])
            ost = out[s].rearrange("(cb c) h w -> c cb (h w)", cb=2)
            if s < S - 1:
                red.tensor_mul(out=xv, in0=xv, in1=awb)
                red.tensor_mul(out=xf, in0=xf, in1=ahf)
                nc.sync.dma_start(out=ost[:, 0], in_=xt[:, 0:HW])
                nc.scalar.dma_start(out=ost[:, 1], in_=xt[:, HW:2 * HW])
            else:
                # last sample: per-(cb, h-half) muls with eighth-tile stores
                for cb in range(2):
                    awc = bass.AP(tensor=a.tensor, offset=aw2[:, cb, :].offset,
                                  ap=[a.ap[0], [0, H // 2], a.ap[2]])
                    for hh in range(2):
                        off = cb * HW + hh * (HW // 2)
                        xvc = xt[:, off:off + HW // 2].rearrange(
                            "p (h w) -> p h w", w=W)
                        red.tensor_mul(out=xvc, in0=xvc, in1=awc)
                        xfc = bass.AP(tensor=xt.tensor,
                                      offset=xt[:, off:].offset,
                                      ap=[xt[:, :].ap[0], [W, H // 2],
                                          [2, W // 2], [1, 2]])
                        ahc = bass.AP(tensor=aa.tensor,
                                      offset=ah2[:, cb, hh * (H // 2):].offset,
                                      ap=[aa.ap[0], [2, H // 2],
                                          [0, W // 2], [1, 2]])
                        red.tensor_mul(out=xfc, in0=xfc, in1=ahc)
                        nq = 4 if (cb == 1 and hh == 1) else 2
                        for qq in range(nq):
                            eng = nc.sync if qq % 2 == 0 else nc.scalar
                            csz = (HW // 2) // nq
                            osl = slice(hh * (HW // 2) + qq * csz,
                                        hh * (HW // 2) + (qq + 1) * csz)
                            eng.dma_start(
                                out=ost[:, cb, osl],
                                in_=xt[:, off + qq * csz:off + (qq + 1) * csz])

        # ---------- schedule: trees in sample order, then muls ----------
        y_tiles = []
        for s in range(S):
            yt = ypool.tile([128, 4, T], f16, tag=f"y{s}")
            y_tiles.append(yt)

        gates_out = [None] * S
        # PE col phases hoisted ahead of conv/gates so the PE never waits
        # on a sample's DVE round-trip before starting the next col block.
        row_tree_half(all_xts[0], y_tiles[0], 0, 0)
        row_tree_half(all_xts[0], y_tiles[0], 0, 1)
        c3_0 = col_pe_mm(all_xts[0])
        row_tree_cb(all_xts[0], y_tiles[0], 1)
        c3_1 = col_pe_mm(all_xts[1])
        col_tail(c3_0, y_tiles[0])
        gates_out[0] = gates(y_tiles[0])
        c3_2 = col_pe_mm(all_xts[2])
        g0 = gate_prefuse(*gates_out[0], 0)
        row_tree_half(all_xts[1], y_tiles[1], 0, 0)
        row_tree_half(all_xts[1], y_tiles[1], 0, 1)
        row_tree_half(all_xts[1], y_tiles[1], 1, 0)
        row_tree_half(all_xts[1], y_tiles[1], 1, 1)
        col_tail(c3_1, y_tiles[1])
        gates_out[1] = gates(y_tiles[1])
        c3_3 = col_pe_mm(all_xts[3])
        g1 = gate_prefuse(*gates_out[1], 1)
        row_tree_half(all_xts[2], y_tiles[2], 0, 0)
        row_tree_half(all_xts[2], y_tiles[2], 0, 1)
        row_tree_half(all_xts[2], y_tiles[2], 1, 0)
        row_tree_half(all_xts[2], y_tiles[2], 1, 1)
        col_tail(c3_2, y_tiles[2])
        gates_out[2] = gates(y_tiles[2])
        muls_fused_store(0, all_xts[0], g0)
        row_tree_joint(all_xts[3], y_tiles[3])
        col_tail(c3_3, y_tiles[3])
        gates_out[3] = gates(y_tiles[3])
        muls_fused_store(1, all_xts[1], g1)
        muls_and_store(2, all_xts[2], *gates_out[2])
        muls_and_store(3, all_xts[3], *gates_out[3])

    if legalize:
        import concourse.mybir as mybir
        _legalize_waits(nc, mybir)
    return nc


def _prep_params(conv1_w, conv1_b, bn_gamma, bn_beta, bn_mean, bn_var,
                 gate_w, gate_b, convh_w, convh_b, convw_w, convw_b):
    f32 = np.float32
    bnscale = (np.asarray(bn_gamma, f32)
               / np.sqrt(np.asarray(bn_var, f32) + BN_EPS)).astype(f32)
    Wc = np.asarray(conv1_w, f32)[:, :, :, 1]
    s_ci = np.where(np.arange(3 * C) < C, 1.0 / W, 2.0 / W).astype(f32)
    W2 = (Wc * s_ci[None, :, None] * bnscale[:, None, None]).astype(f32)
    bias2 = ((np.asarray(conv1_b, f32) - np.asarray(bn_mean, f32)) * bnscale
             + np.asarray(bn_beta, f32)).astype(f32)
    W6 = W2.reshape(MIP, 6, 128, 3)
    W4 = np.stack([W6[:, 2 + gp] + W6[:, gp % 2] for gp in range(4)], axis=1)
    w2 = np.ascontiguousarray(W4.transpose(2, 1, 3, 0)).astype(np.float16)
    gw = np.full((MIP, 1), float(gate_w) / T, f32)
    gb = np.full((MIP, 1), float(gate_b), f32)
    wh = np.ascontiguousarray(np.asarray(convh_w, np.float16).T)
    ww = np.ascontiguousarray(np.asarray(convw_w, np.float16).T)
    bh = np.ascontiguousarray(np.asarray(convh_b, f32).reshape(2, 128).T)
    bw = np.ascontiguousarray(np.asarray(convw_b, f32).reshape(2, 128).T)
    idn = np.ascontiguousarray(np.eye(128, dtype=np.float16))
    return dict(w2=w2, bias2=bias2.reshape(MIP, 1), gwv=gw, gbv=gb,
                wh=wh, ww=ww, bh=bh, bw=bw, idn=idn)


def kernel(**inputs):
    import sys
    if "/opt/trn_rl_repo" not in sys.path:
        sys.path.insert(0, "/opt/trn_rl_repo")
    from concourse.bass_utils import run_bass_kernel_spmd

    x = np.asarray(inputs["x"], np.float32).astype(np.float16)
    params = _prep_params(
        inputs["conv1_w"], inputs["conv1_b"], inputs["bn_gamma"],
        inputs["bn_beta"], inputs["bn_mean"], inputs["bn_var"],
        inputs["gate_w"], inputs["gate_b"], inputs["convh_w"],
        inputs["convh_b"], inputs["convw_w"], inputs["convw_b"])

    if "nc" not in _CACHE:
        _CACHE["nc"] = _build_program()
    nc = _CACHE["nc"]

    in_maps = [
        {"xs": np.ascontiguousarray(x[i * S:(i + 1) * S]), **params}
        for i in range(N_CORES)
    ]
    res = run_bass_kernel_spmd(nc, in_maps, core_ids=list(range(N_CORES)))
    out = np.concatenate([r["out"] for r in res.results], axis=0)
    return out.astype(np.float32)


# revision 11
# speedup vs baseline: 1.0568x; 1.0012x over previous
"""Trainium2 Bass kernel for nn_EnhancedCoordAtt — v3.

vs baseline (97us -> ~90us):
  - h-gate x2-replication instead of full ACT expansion: ACT writes
    ah2[p,cb,h,0:2] (256 elems, ~0.4us instead of 27us); mulB's in1 AP
    [(cb h) stride-2][w_hi 0-stride][w_lo unit-2] keeps the innermost
    dim unit-stride/2-elem, which is all the DVE 2x_1p mode needs
    (HW-verified 4426ns for [128,8192], same as the full expansion).
  - col-pooling levels L1-L3 (8-row partial sums) moved to the PE as
    identity-lhsT accumulated matmuls into PSUM (16 mm of f=512 per
    sample, contiguous-64 rhs inner dim, fp32-exact), ACT evacuates and
    casts to fp16; DVE only does the last two levels (-3.3us/sample).
    The row tree stays on DVE: its reduction axis IS the contiguous w,
    so PE rhs slices would have 8-elem strided runs (measured 2x cost).
  - PE col phases hoisted ahead of conv/gate matmuls in program order so
    the PE never waits on a sample's DVE round-trip before starting the
    next col block (this PE drag taxed all concurrent DVE muls ~18%).
  - loads/stores striped over both HWDGE rings (single-ring DMA was
    measured to slow concurrent DVE ops by ~18%).
  - s0 trees chunked (cb0 h-halves first) for earliest start; last
    sample's muls/stores chunked per (cb, h-half) for a short tail.
  - gate prefuse for s0/s1: g = aw*ah2 is combined during the DVE's
    load-bound idle windows (~27-33us), so their gate application in the
    packed phase is a single x*g pass instead of two.  (Prefusing the
    later samples was measured neutral-to-worse: their gates only become
    ready mid mul-phase, so the prefuse displaces muls instead of
    filling idle time.)
All shapes hardcoded to the nn_EnhancedCoordAtt_78855599555233 spec.
"""

import numpy as np

N, C, H, W = 32, 256, 64, 64
MIP = 8
N_CORES = 8
S = N // N_CORES
HW = H * W
T = H + W
BN_EPS = 1e-5

_CACHE = {}


def _legalize_waits(nc, mybir, max_keep=1):
    f = nc.m.functions[0]
    for blk in f.blocks:
        out, changed = [], False
        for inst in blk.instructions:
            si = inst.sync_info
            t = type(inst).__name__
            if (si is not None and len(si.on_wait) > max_keep
                    and t != "InstEventSemaphore"):
                waits = list(si.on_wait)
                for j, w in enumerate(waits[:-max_keep]):
                    ev = mybir.InstEventSemaphore(
                        name=f"{inst.name}_xw{j}", ins=[], outs=[])
                    ev.engine = inst.engine
                    ev.sync_info = mybir.SyncInfo(on_wait=[w], on_update=[])
                    out.append(ev)
                inst.sync_info = mybir.SyncInfo(
                    on_wait=waits[-max_keep:], on_update=list(si.on_update))
                changed = True
            out.append(inst)
        if changed:
            blk.instructions = out


def _build_program(legalize=True, sim_compat=False):
    import concourse.bass as bass
    import concourse.tile as tile
    import concourse.mybir as mybir
    from contextlib import ExitStack

    f16 = mybir.dt.float16
    f32 = mybir.dt.float32
    i32 = mybir.dt.int32
    nc = bass.Bass()

    xs = nc.declare_dram_parameter("xs", [S, C, H, W], f16, isOutput=False)
    w2 = nc.declare_dram_parameter("w2", [128, 4, 3, MIP], f16, isOutput=False)
    bias2 = nc.declare_dram_parameter("bias2", [MIP, 1], f32, isOutput=False)
    gwv = nc.declare_dram_parameter("gwv", [MIP, 1], f32, isOutput=False)
    gbv = nc.declare_dram_parameter("gbv", [MIP, 1], f32, isOutput=False)
    wh = nc.declare_dram_parameter("wh", [MIP, C], f16, isOutput=False)
    ww = nc.declare_dram_parameter("ww", [MIP, C], f16, isOutput=False)
    bh = nc.declare_dram_parameter("bh", [128, 2], f32, isOutput=False)
    bw = nc.declare_dram_parameter("bw", [128, 2], f32, isOutput=False)
    idn = nc.declare_dram_parameter("idn", [128, 128], f16, isOutput=False)
    out = nc.declare_dram_parameter("out", [S, C, H, W], f16, isOutput=True)

    with tile.TileContext(nc) as tc, ExitStack() as ctx:
        ctx.enter_context(nc.allow_low_precision(reason="2e-2 tolerance, fp16 path"))
        red = nc.vector
        Sig = mybir.ActivationFunctionType.Sigmoid
        Copy = mybir.ActivationFunctionType.Copy

        singles = ctx.enter_context(tc.tile_pool(name="singles", bufs=1))
        xpool = ctx.enter_context(tc.tile_pool(name="xin", bufs=4))
        ypool = ctx.enter_context(tc.tile_pool(name="yall", bufs=4))
        small = ctx.enter_context(tc.tile_pool(name="small", bufs=4))
        tpool = ctx.enter_context(tc.tile_pool(name="tree", bufs=2))
        cpool = ctx.enter_context(tc.tile_pool(name="colp", bufs=4))
        apool = ctx.enter_context(tc.tile_pool(name="attn", bufs=4))
        gpool = ctx.enter_context(tc.tile_pool(name="gfuse", bufs=1))
        pspool = ctx.enter_context(tc.tile_pool(name="ps", bufs=2, space="PSUM"))
        psgate = ctx.enter_context(tc.tile_pool(name="psg", bufs=1, space="PSUM"))
        pscol = ctx.enter_context(tc.tile_pool(name="psc", bufs=4, space="PSUM"))

        # ---- identity matrix for PE copy/accumulate matmuls (host param) ----
        ident = singles.tile([128, 128], f16)
        nc.gpsimd.dma_start(out=ident, in_=idn[:, :])

        # ---- params on gpsimd SWDGE (never queues behind x traffic) ----
        w2sb = singles.tile([128, 4, 3, MIP], f16)
        nc.gpsimd.dma_start(out=w2sb, in_=w2[:, :, :, :])
        bias2sb = singles.tile([MIP, 1], f32)
        nc.gpsimd.dma_start(out=bias2sb, in_=bias2[:, :])
        gwsb = singles.tile([MIP, 1], f32)
        nc.gpsimd.dma_start(out=gwsb, in_=gwv[:, :])
        gbsb = singles.tile([MIP, 1], f32)
        nc.gpsimd.dma_start(out=gbsb, in_=gbv[:, :])
        whsb = singles.tile([MIP, C], f16)
        nc.gpsimd.dma_start(out=whsb, in_=wh[:, :])
        wwsb = singles.tile([MIP, C], f16)
        nc.gpsimd.dma_start(out=wwsb, in_=ww[:, :])
        bhsb = singles.tile([128, 2], f32)
        nc.gpsimd.dma_start(out=bhsb, in_=bh[:, :])
        bwsb = singles.tile([128, 2], f32)
        nc.gpsimd.dma_start(out=bwsb, in_=bw[:, :])

        # ---- x loads, striped across both rings ----
        all_xts = []
        for s in range(S):
            xt = xpool.tile([128, 2 * HW], f16, tag="xt")
            all_xts.append(xt)
            src_full = xs[s].rearrange("(cb c) h w -> c cb (h w)", cb=2)
            if s == 0:
                # cb0 as 0.25MB h-quarters (earliest possible first tree op)
                for q in range(4):
                    eng = nc.sync if q < 2 else nc.scalar
                    sl = slice(q * (HW // 4), (q + 1) * (HW // 4))
                    eng.dma_start(out=xt[:, sl], in_=src_full[:, 0, sl])
                for hh in range(2):
                    eng = nc.sync if hh == 0 else nc.scalar
                    sl = slice(HW + hh * (HW // 2), HW + (hh + 1) * (HW // 2))
                    eng.dma_start(
                        out=xt[:, sl],
                        in_=src_full[:, 1, hh * (HW // 2):(hh + 1) * (HW // 2)])
            elif s <= 2:
                # per-(cb, h-half) chunks on both rings: finer arrival
                # granularity lets early-sample tree work start sooner
                for cb in range(2):
                    for hh in range(2):
                        eng = nc.sync if hh == 0 else nc.scalar
                        sl = slice(cb * HW + hh * (HW // 2),
                                   cb * HW + (hh + 1) * (HW // 2))
                        eng.dma_start(
                            out=xt[:, sl],
                            in_=src_full[:, cb, hh * (HW // 2):(hh + 1) * (HW // 2)])
            else:
                for cb in range(2):
                    eng = nc.sync if cb == 0 else nc.scalar
                    eng.dma_start(
                        out=xt[:, cb * HW:(cb + 1) * HW], in_=src_full[:, cb])

        def row_tree_q(xt, y_all, cb, q):
            """Row-half sums for one (cb, h-quarter): h in [q*16,(q+1)*16)."""
            xh = xt[:, cb * HW + q * (HW // 4): cb * HW + (q + 1) * (HW // 4)]
            rv = xh.rearrange("p (hj w) -> p hj w", w=W // 2)
            r1 = tpool.tile([128, H // 2, 16], f16, tag="qr1")
            red.tensor_add(out=r1, in0=rv[:, :, 0:16], in1=rv[:, :, 16:32])
            r2 = tpool.tile([128, H // 2, 8], f16, tag="qr2")
            red.tensor_add(out=r2, in0=r1[:, :, 0:8], in1=r1[:, :, 8:16])
            r3 = tpool.tile([128, H // 2, 4], f16, tag="qr3")
            red.tensor_add(out=r3, in0=r2[:, :, 0:4], in1=r2[:, :, 4:8])
            r4 = tpool.tile([128, H // 2, 2], f16, tag="qr4")
            red.tensor_add(out=r4, in0=r3[:, :, 0:2], in1=r3[:, :, 2:4])
            rv4 = r4.rearrange("p (h j) a -> p j h a", j=2)
            red.tensor_add(
                out=y_all.rearrange("p (j c) t -> p c j t", j=2)[
                    :, cb, :, q * (H // 4):(q + 1) * (H // 4)],
                in0=rv4[:, :, :, 0], in1=rv4[:, :, :, 1])

        def row_tree_half(xt, y_all, cb, i):
            """Row-half sums for one (cb, h-half) chunk (earliest start)."""
            xh = xt[:, cb * HW + i * (HW // 2): cb * HW + (i + 1) * (HW // 2)]
            rv = xh.rearrange("p (hj w) -> p hj w", w=W // 2)
            r1 = tpool.tile([128, H, 16], f16, tag="hr1")
            red.tensor_add(out=r1, in0=rv[:, :, 0:16], in1=rv[:, :, 16:32])
            r2 = tpool.tile([128, H, 8], f16, tag="hr2")
            red.tensor_add(out=r2, in0=r1[:, :, 0:8], in1=r1[:, :, 8:16])
            r3 = tpool.tile([128, H, 4], f16, tag="hr3")
            red.tensor_add(out=r3, in0=r2[:, :, 0:4], in1=r2[:, :, 4:8])
            r4 = tpool.tile([128, H, 2], f16, tag="hr4")
            red.tensor_add(out=r4, in0=r3[:, :, 0:2], in1=r3[:, :, 2:4])
            rv4 = r4.rearrange("p (h j) a -> p j h a", j=2)
            red.tensor_add(
                out=y_all.rearrange("p (j c) t -> p c j t", j=2)[
                    :, cb, :, i * (H // 2):(i + 1) * (H // 2)],
                in0=rv4[:, :, :, 0], in1=rv4[:, :, :, 1])

        def row_tree_cb(xt, y_all, cb):
            """Row-half sums for one cb block (s0 path: starts on cb0)."""
            xc = xt[:, cb * HW:(cb + 1) * HW]
            rv = xc.rearrange("p (hj w) -> p hj w", w=W // 2)
            r1 = tpool.tile([128, 2 * H, 16], f16, tag="r1")
            red.tensor_add(out=r1, in0=rv[:, :, 0:16], in1=rv[:, :, 16:32])
            r2 = tpool.tile([128, 2 * H, 8], f16, tag="r2")
            red.tensor_add(out=r2, in0=r1[:, :, 0:8], in1=r1[:, :, 8:16])
            r3 = tpool.tile([128, 2 * H, 4], f16, tag="r3")
            red.tensor_add(out=r3, in0=r2[:, :, 0:4], in1=r2[:, :, 4:8])
            r4 = tpool.tile([128, 2 * H, 2], f16, tag="r4")
            red.tensor_add(out=r4, in0=r3[:, :, 0:2], in1=r3[:, :, 2:4])
            rv4 = r4.rearrange("p (h j) a -> p j h a", j=2)
            red.tensor_add(
                out=y_all.rearrange("p (j c) t -> p c j t", j=2)[:, cb, :, 0:H],
                in0=rv4[:, :, :, 0], in1=rv4[:, :, :, 1])

        def row_tree_joint(xt, y_all):
            rv = xt.rearrange("p (cb hj w) -> p cb hj w", cb=2, w=W // 2)
            r1 = tpool.tile([128, 2, 2 * H, 16], f16, tag="jr1")
            red.tensor_add(out=r1, in0=rv[:, :, :, 0:16], in1=rv[:, :, :, 16:32])
            r2 = tpool.tile([128, 2, 2 * H, 8], f16, tag="jr2")
            red.tensor_add(out=r2, in0=r1[:, :, :, 0:8], in1=r1[:, :, :, 8:16])
            r3 = tpool.tile([128, 2, 2 * H, 4], f16, tag="jr3")
            red.tensor_add(out=r3, in0=r2[:, :, :, 0:4], in1=r2[:, :, :, 4:8])
            r4 = tpool.tile([128, 2, 2 * H, 2], f16, tag="jr4")
            red.tensor_add(out=r4, in0=r3[:, :, :, 0:2], in1=r3[:, :, :, 2:4])
            rv4 = r4.rearrange("p cb (h j) a -> p cb j h a", j=2)
            red.tensor_add(
                out=y_all.rearrange("p (j c) t -> p c j t", j=2)[:, :, :, 0:H],
                in0=rv4[:, :, :, :, 0], in1=rv4[:, :, :, :, 1])

        def col_pe_mm(xt):
            """Col L1-L3 (8-row partials) on PE + ACT evac/cast, per cb."""
            c3es = []
            for cb in range(2):
                psc = pscol.tile([128, 8, W], f32, tag="psc")
                for r in range(8):
                    base = xt[:, cb * HW + r * W:]
                    rhs = bass.AP(tensor=base.tensor, offset=base.offset,
                                  ap=[base.ap[0], [8 * W, 8], [1, W]])
                    nc.tensor.matmul(out=psc, lhsT=ident[:, :], rhs=rhs,
                                     start=(r == 0), stop=(r == 7))
                c3e = cpool.tile([128, 8, W], f16, tag="c3e")
                nc.scalar.activation(out=c3e, in_=psc, func=Copy,
                                     bias=0.0, scale=1.0)
                c3es.append(c3e)
            return c3es

        def col_tail(c3es, y_all):
            """Last two col levels on DVE."""
            for cb in range(2):
                c3e = c3es[cb]
                c4 = cpool.tile([128, 2, 2, W], f16, tag="c4")
                cv = c3e.rearrange("p (i m) w -> p i m w", i=2)
                red.tensor_add(out=c4, in0=cv[:, :, 0:2], in1=cv[:, :, 2:4])
                red.tensor_add(
                    out=y_all.rearrange("p (i c) t -> p i c t", i=2)[:, :, cb, H:T],
                    in0=c4[:, :, 0, :], in1=c4[:, :, 1, :])

        def gates(y_all):
            psy = pspool.tile([MIP, T], f32, tag="psy")
            order = [(0, 1)] + [(g, k) for g in range(4) for k in range(3)
                                if (g, k) != (0, 1)]
            for idx, (g, k) in enumerate(order):
                lhsT = w2sb[:, g, k, :]
                if k == 1:
                    o_sl, i_sl = slice(0, T), slice(0, T)
                elif k == 0:
                    o_sl, i_sl = slice(2, T), slice(0, T - 2)
                else:
                    o_sl, i_sl = slice(0, T - 2), slice(2, T)
                nc.tensor.matmul(
                    out=psy[:, o_sl], lhsT=lhsT, rhs=y_all[:, g, i_sl],
                    start=(idx == 0), stop=(idx == len(order) - 1))

            ya0 = small.tile([MIP, T], f32, tag="ya0")
            nc.vector.tensor_scalar_add(out=ya0, in0=psy, scalar1=bias2sb[:, :])
            ysg = small.tile([MIP, T], f32, tag="ysg")
            nc.scalar.activation(out=ysg, in_=ya0, func=Sig, bias=0.0, scale=1.0)
            ya = small.tile([MIP, T], f32, tag="ya")
            red.tensor_mul(out=ya, in0=ya0, in1=ysg)
            ysum = small.tile([MIP, 1], f32, tag="ysum")
            red.reduce_sum(out=ysum, in_=ya, axis=mybir.AxisListType.X)
            se = small.tile([MIP, 1], f32, tag="se")
            nc.scalar.activation(out=se, in_=ysum, func=Sig,
                                 bias=gbsb[:, :], scale=gwsb[:, :])
            yg = small.tile([MIP, T], f16, tag="yg")
            nc.vector.tensor_scalar_mul(out=yg, in0=ya, scalar1=se[:, :])

            ah2 = apool.tile([128, 2, H, 2], f16, tag="ah2")
            aw2 = apool.tile([128, 2, W], f16, tag="aw2")
            for cb in range(2):
                psa = psgate.tile([128, H], f32, tag="psa")
                nc.tensor.matmul(
                    out=psa, lhsT=whsb[:, cb * 128:(cb + 1) * 128],
                    rhs=yg[:, 0:H], start=True, stop=True)
                pa = psa[:, :]
                pab = bass.AP(tensor=pa.tensor, offset=pa.offset,
                              ap=[pa.ap[0], pa.ap[1], [0, 2]])
                nc.scalar.activation(out=ah2[:, cb], in_=pab, func=Sig,
                                     bias=bhsb[:, cb:cb + 1], scale=1.0)
                psb = psgate.tile([128, W], f32, tag="psb")
                nc.tensor.matmul(
                    out=psb, lhsT=wwsb[:, cb * 128:(cb + 1) * 128],
                    rhs=yg[:, H:T], start=True, stop=True)
                nc.scalar.activation(out=aw2[:, cb], in_=psb, func=Sig,
                                     bias=bwsb[:, cb:cb + 1], scale=1.0)
            return ah2, aw2

        def gate_prefuse(ah2, aw2, s):
            """Combine g = aw * ah2 into a full fp16 tile during DVE idle
            (load-bound) windows; the mul phase then needs ONE pass."""
            g = gpool.tile([128, 2 * HW], f16, tag=f"g{s}")
            aa = ah2[:, :, :, :]
            a = aw2[:, :, :]
            for cb in range(2):
                gout = bass.AP(tensor=g.tensor, offset=g[:, cb * HW:].offset,
                               ap=[g[:, :].ap[0], [W, H], [2, W // 2], [1, 2]])
                gin0 = bass.AP(tensor=a.tensor, offset=aw2[:, cb, :].offset,
                               ap=[a.ap[0], [0, H], [2, W // 2], [1, 2]])
                gin1 = bass.AP(tensor=aa.tensor, offset=ah2[:, cb].offset,
                               ap=[aa.ap[0], [2, H], [0, W // 2], [1, 2]])
                red.tensor_mul(out=gout, in0=gin0, in1=gin1)
            return g

        def muls_fused_store(s, xt, g):
            """out = x * g in one DVE pass, then store."""
            red.tensor_mul(out=xt[:, :], in0=xt[:, :], in1=g[:, :])
            ost = out[s].rearrange("(cb c) h w -> c cb (h w)", cb=2)
            nc.sync.dma_start(out=ost[:, 0], in_=xt[:, 0:HW])
            nc.scalar.dma_start(out=ost[:, 1], in_=xt[:, HW:2 * HW])

        def muls_and_store(s, xt, ah2, aw2):
            xv = xt.rearrange("p (cb h w) -> p cb h w", cb=2, w=W)
            a = aw2[:, :, :]
            awb = bass.AP(tensor=a.tensor, offset=a.offset,
                          ap=[a.ap[0], a.ap[1], [0, H], a.ap[2]])
            aa = ah2[:, :, :, :]
            ahf = bass.AP(tensor=aa.tensor, offset=aa.offset,
                          ap=[aa.ap[0], [2, 128], [0, W // 2], [1, 2]])
            xf = bass.AP(tensor=xt.tensor, offset=xt[:, :].offset,
                         ap=[xt[:, :].ap[0], [W, 2 * H], [2, W // 2], [1, 2]])
            ost = out[s].rearrange("(cb c) h w -> c cb (h w)", cb=2)
            if s < S - 1:
                red.tensor_mul(out=xv, in0=xv, in1=awb)
                red.tensor_mul(out=xf, in0=xf, in1=ahf)
                nc.sync.dma_start(out=ost[:, 0], in_=xt[:, 0:HW])
                nc.scalar.dma_start(out=ost[:, 1], in_=xt[:, HW:2 * HW])
            else:
                # last sample: per-(cb, h-half) muls with eighth-tile stores
                for cb in range(2):
                    awc = bass.AP(tensor=a.tensor, offset=aw2[:, cb, :].offset,
                                  ap=[a.ap[0], [0, H // 2], a.ap[2]])
                    for hh in range(2):
                        off = cb * HW + hh * (HW // 2)
                        xvc = xt[:, off:off + HW // 2].rearrange(
                            "p (h w) -> p h w", w=W)
                        red.tensor_mul(out=xvc, in0=xvc, in1=awc)
                        xfc = bass.AP(tensor=xt.tensor,
                                      offset=xt[:, off:].offset,
                                      ap=[xt[:, :].ap[0], [W, H // 2],
                                          [2, W // 2], [1, 2]])
                        ahc = bass.AP(tensor=aa.tensor,
                                      offset=ah2[:, cb, hh * (H // 2):].offset,
                                      ap=[aa.ap[0], [2, H // 2],
                                          [0, W // 2], [1, 2]])
                        red.tensor_mul(out=xfc, in0=xfc, in1=ahc)
                        nq = 4 if (cb == 1 and hh == 1) else 2
                        for qq in range(nq):
                            eng = nc.sync if qq % 2 == 0 else nc.scalar
                            csz = (HW // 2) // nq
                            osl = slice(hh * (HW // 2) + qq * csz,
                                        hh * (HW // 2) + (qq + 1) * csz)
                            eng.dma_start(
                                out=ost[:, cb, osl],
                                in_=xt[:, off + qq * csz:off + (qq + 1) * csz])

        # ---------- schedule: trees in sample order, then muls ----------
        y_tiles = []
        for s in range(S):
            yt = ypool.tile([128, 4, T], f16, tag=f"y{s}")
            y_tiles.append(yt)

        gates_out = [None] * S
        # PE col phases hoisted ahead of conv/gates so the PE never waits
        # on a sample's DVE round-trip before starting the next col block.
        for q in range(4):
            row_tree_q(all_xts[0], y_tiles[0], 0, q)
        c3_0 = col_pe_mm(all_xts[0])
        row_tree_cb(all_xts[0], y_tiles[0], 1)
        c3_1 = col_pe_mm(all_xts[1])
        col_tail(c3_0, y_tiles[0])
        gates_out[0] = gates(y_tiles[0])
        c3_2 = col_pe_mm(all_xts[2])
        g0 = gate_prefuse(*gates_out[0], 0)
        row_tree_half(all_xts[1], y_tiles[1], 0, 0)
        row_tree_half(all_xts[1], y_tiles[1], 0, 1)
        row_tree_half(all_xts[1], y_tiles[1], 1, 0)
        row_tree_half(all_xts[1], y_tiles[1], 1, 1)
        col_tail(c3_1, y_tiles[1])
        gates_out[1] = gates(y_tiles[1])
        c3_3 = col_pe_mm(all_xts[3])
        g1 = gate_prefuse(*gates_out[1], 1)
        row_tree_half(all_xts[2], y_tiles[2], 0, 0)
        row_tree_half(all_xts[2], y_tiles[2], 0, 1)
        row_tree_half(all_xts[2], y_tiles[2], 1, 0)
        row_tree_half(all_xts[2], y_tiles[2], 1, 1)
        col_tail(c3_2, y_tiles[2])
        gates_out[2] = gates(y_tiles[2])
        muls_fused_store(0, all_xts[0], g0)
        row_tree_joint(all_xts[3], y_tiles[3])
        col_tail(c3_3, y_tiles[3])
        gates_out[3] = gates(y_tiles[3])
        muls_fused_store(1, all_xts[1], g1)
        muls_and_store(2, all_xts[2], *gates_out[2])
        muls_and_store(3, all_xts[3], *gates_out[3])

    if legalize:
        import concourse.mybir as mybir
        _legalize_waits(nc, mybir)
    return nc


def _prep_params(conv1_w, conv1_b, bn_gamma, bn_beta, bn_mean, bn_var,
                 gate_w, gate_b, convh_w, convh_b, convw_w, convw_b):
    f32 = np.float32
    bnscale = (np.asarray(bn_gamma, f32)
               / np.sqrt(np.asarray(bn_var, f32) + BN_EPS)).astype(f32)
    Wc = np.asarray(conv1_w, f32)[:, :, :, 1]
    s_ci = np.where(np.arange(3 * C) < C, 1.0 / W, 2.0 / W).astype(f32)
    W2 = (Wc * s_ci[None, :, None] * bnscale[:, None, None]).astype(f32)
    bias2 = ((np.asarray(conv1_b, f32) - np.asarray(bn_mean, f32)) * bnscale
             + np.asarray(bn_beta, f32)).astype(f32)
    W6 = W2.reshape(MIP, 6, 128, 3)
    W4 = np.stack([W6[:, 2 + gp] + W6[:, gp % 2] for gp in range(4)], axis=1)
    w2 = np.ascontiguousarray(W4.transpose(2, 1, 3, 0)).astype(np.float16)
    gw = np.full((MIP, 1), float(gate_w) / T, f32)
    gb = np.full((MIP, 1), float(gate_b), f32)
    wh = np.ascontiguousarray(np.asarray(convh_w, np.float16).T)
    ww = np.ascontiguousarray(np.asarray(convw_w, np.float16).T)
    bh = np.ascontiguousarray(np.asarray(convh_b, f32).reshape(2, 128).T)
    bw = np.ascontiguousarray(np.asarray(convw_b, f32).reshape(2, 128).T)
    idn = np.ascontiguousarray(np.eye(128, dtype=np.float16))
    return dict(w2=w2, bias2=bias2.reshape(MIP, 1), gwv=gw, gbv=gb,
                wh=wh, ww=ww, bh=bh, bw=bw, idn=idn)


def kernel(**inputs):
    import sys
    if "/opt/trn_rl_repo" not in sys.path:
        sys.path.insert(0, "/opt/trn_rl_repo")
    from concourse.bass_utils import run_bass_kernel_spmd

    x = np.asarray(inputs["x"], np.float32).astype(np.float16)
    params = _prep_params(
        inputs["conv1_w"], inputs["conv1_b"], inputs["bn_gamma"],
        inputs["bn_beta"], inputs["bn_mean"], inputs["bn_var"],
        inputs["gate_w"], inputs["gate_b"], inputs["convh_w"],
        inputs["convh_b"], inputs["convw_w"], inputs["convw_b"])

    if "nc" not in _CACHE:
        _CACHE["nc"] = _build_program()
    nc = _CACHE["nc"]

    in_maps = [
        {"xs": np.ascontiguousarray(x[i * S:(i + 1) * S]), **params}
        for i in range(N_CORES)
    ]
    res = run_bass_kernel_spmd(nc, in_maps, core_ids=list(range(N_CORES)))
    out = np.concatenate([r["out"] for r in res.results], axis=0)
    return out.astype(np.float32)
